# revision 14
# baseline (speedup 1.0000x reference)
"""Trainium2 Bass kernel for nn_BasicTransformerBlock (self-attn + cross-attn + GEGLU).

Sharding: data-parallel over the 2048 tokens (256 per core, 8 cores, no
collectives). K/V for self-attention are computed replicated on every core.

On-chip layout is feature-major throughout ([feature(part), token(free)]).
Host pre-packs weights as bf16 W.T (C-contiguous [in, out]) and pre-transposes
x / context, so the device does zero transposes/casts and all DMAs are
contiguous. Weight/projection matmuls run in bf16 (fp32 PSUM accumulate);
LayerNorm statistics run in float32r off the fp32 residual stream.

Softmax: scores are computed keys-on-partitions ([keys, q]); exp on ScalarE
with the 1/8 scale folded in; denominators come from an appended ones-column
in V (row 64 of the AV accumulation); all 16 head sums are staged into one
tile, inverted with a single reciprocal, and divided in via a PE-broadcast.
"""

import numpy as np
import ml_dtypes

import concourse.bass as bass
import concourse.mybir as mybir
import concourse.tile as tile
from concourse import bacc
from concourse.bass_utils import run_bass_kernel_spmd

F32 = mybir.dt.float32
F32R = mybir.dt.float32r
BF16 = mybir.dt.bfloat16
AF = mybir.ActivationFunctionType
OP = mybir.AluOpType

P = 128
N, D = 2048, 1024
H, DH = 16, 64
CN, CD = 77, 768
FF = 4096
EPS = 1e-5
SCALE = DH ** -0.5
NCORES = 8
TO = N // NCORES          # 256 tokens owned per core
DT = D // P               # 8 feature tiles
CT = CD // P              # 6 context-feature tiles
NKT = N // P              # 16 key tiles
FT = FF // P              # 32 ffn-inner tiles


def _ln_feature_major(nc, lnp, sbp, consts, src_of, dst_of, n_dt, tn, chunk):
    """Un-affine LayerNorm over feature-major f32r data."""
    ones_col, ones_row, eps_t = consts
    inv_d = 1.0 / (n_dt * P)
    for tci in range(tn // chunk):
        srcs = [src_of(dt, tci) for dt in range(n_dt)]   # f32r tiles
        sum_ps = lnp.tile([1, chunk], F32, tag="ln_sum", bufs=2)
        for dt in range(n_dt):
            nc.tensor.matmul(sum_ps, ones_col, srcs[dt],
                             start=(dt == 0), stop=(dt == n_dt - 1))
        sumsq_ps = lnp.tile([1, chunk], F32, tag="ln_sumsq", bufs=2)
        for dt in range(n_dt):
            sq_t = sbp.tile([P, chunk], F32R, tag="ln_sq", bufs=2)
            nc.scalar.activation(sq_t, srcs[dt].bitcast(F32), AF.Square)
            nc.tensor.matmul(sumsq_ps, ones_col, sq_t,
                             start=(dt == 0), stop=(dt == n_dt - 1))
        mu_row = sbp.tile([1, chunk], F32R, tag="ln_mu", bufs=2)
        nc.scalar.mul(out=mu_row, in_=sum_ps, mul=inv_d)
        var_row = sbp.tile([1, chunk], F32, tag="ln_var", bufs=2)
        nc.scalar.mul(out=var_row, in_=sumsq_ps, mul=inv_d)
        musq = sbp.tile([1, chunk], F32, tag="ln_musq", bufs=2)
        nc.vector.tensor_mul(out=musq, in0=mu_row.bitcast(F32),
                             in1=mu_row.bitcast(F32))
        nc.vector.tensor_tensor(out=var_row, in0=var_row, in1=musq,
                                op=OP.subtract)
        nc.scalar.activation(var_row, var_row, AF.Sqrt, bias=eps_t)
        rstd_row = sbp.tile([1, chunk], F32R, tag="ln_rstd", bufs=2)
        with nc.allow_low_precision("f32r keeps full fp32 bits here"):
            nc.vector.reciprocal(rstd_row, var_row)
        mu_b = lnp.tile([P, chunk], F32, tag="ln_mub", bufs=2)
        nc.tensor.matmul(mu_b, ones_row, mu_row, start=True, stop=True)
        rstd_b = lnp.tile([P, chunk], F32, tag="ln_rstdb", bufs=2)
        nc.tensor.matmul(rstd_b, ones_row, rstd_row, start=True, stop=True)
        for dt in range(n_dt):
            tmp = sbp.tile([P, chunk], F32, tag="ln_tmp", bufs=2)
            nc.vector.tensor_tensor(out=tmp, in0=srcs[dt].bitcast(F32),
                                    in1=mu_b, op=OP.subtract)
            nc.vector.tensor_tensor(out=dst_of(dt, tci), in0=tmp, in1=rstd_b,
                                    op=OP.mult)


def build(flags):
    has_qkv1b, has_bo1, has_q2b, has_bo2, has_gegb, has_ffb = flags
    nc = bacc.Bacc()

    xT = nc.dram_tensor("xT", [D, N], F32R, kind="ExternalInput")
    xoT = nc.dram_tensor("xoT", [D, TO], F32R, kind="ExternalInput")
    ctxT = nc.dram_tensor("ctxT", [CD, CN], BF16, kind="ExternalInput")
    wq1T = nc.dram_tensor("wq1T", [D, D], BF16, kind="ExternalInput")
    wk1T = nc.dram_tensor("wk1T", [D, D], BF16, kind="ExternalInput")
    wv1T = nc.dram_tensor("wv1T", [D, D], BF16, kind="ExternalInput")
    wo1T = nc.dram_tensor("wo1T", [D, D], BF16, kind="ExternalInput")
    wq2T = nc.dram_tensor("wq2T", [D, D], BF16, kind="ExternalInput")
    wk2T = nc.dram_tensor("wk2T", [CD, D], BF16, kind="ExternalInput")
    wv2T = nc.dram_tensor("wv2T", [CD, D], BF16, kind="ExternalInput")
    wo2T = nc.dram_tensor("wo2T", [D, D], BF16, kind="ExternalInput")
    wgT = nc.dram_tensor("wgT", [D, 2 * FF], BF16, kind="ExternalInput")
    wfT = nc.dram_tensor("wfT", [FF, D], BF16, kind="ExternalInput")
    onesc = nc.dram_tensor("onesc", [P, 1], F32R, kind="ExternalInput")
    onesr = nc.dram_tensor("onesr", [1, P], F32R, kind="ExternalInput")
    onesb = nc.dram_tensor("onesb", [1, 512], BF16, kind="ExternalInput")
    bias_rows = {}
    if has_qkv1b:
        for nm in ("bq1", "bk1", "bv1"):
            bias_rows[nm] = nc.dram_tensor(nm, [1, D], BF16, kind="ExternalInput")
    if has_bo1:
        bias_rows["bo1"] = nc.dram_tensor("bo1", [1, D], BF16, kind="ExternalInput")
    if has_q2b:
        bias_rows["bq2"] = nc.dram_tensor("bq2", [1, D], BF16, kind="ExternalInput")
    if has_bo2:
        bias_rows["bo2"] = nc.dram_tensor("bo2", [1, D], BF16, kind="ExternalInput")
    if has_gegb:
        bias_rows["bgeg"] = nc.dram_tensor("bgeg", [1, 2 * FF], BF16,
                                           kind="ExternalInput")
    if has_ffb:
        bias_rows["bff"] = nc.dram_tensor("bff", [1, D], BF16, kind="ExternalInput")
    yT = nc.dram_tensor("yT", [D, TO], F32R, kind="ExternalOutput")

    xT_v = xT.rearrange("(dt p) t -> dt p t", p=P)
    xoT_v = xoT.rearrange("(dt p) t -> dt p t", p=P)
    ctxT_v = ctxT.rearrange("(ct p) t -> ct p t", p=P)
    yT_v = yT.rearrange("(dt p) t -> p dt t", p=P)

    def wview(w):
        return w.rearrange("(it p) o -> p it o", p=P)

    with tile.TileContext(nc) as tc:
        with tc.tile_pool(name="consts", bufs=1) as cpool, \
             tc.tile_pool(name="pers", bufs=1) as pers, \
             tc.tile_pool(name="wmain", bufs=1) as wmain:

            ones_col = cpool.tile([P, 1], F32R)
            nc.sync.dma_start(ones_col, onesc[:])
            ones_row = cpool.tile([1, P], F32R)
            nc.sync.dma_start(ones_row, onesr[:])
            ones_b = cpool.tile([1, 512], BF16)
            nc.sync.dma_start(ones_b, onesb[:])
            eps_t = cpool.tile([1, 1], F32)
            nc.vector.memset(eps_t, EPS)
            consts = (ones_col, ones_row, eps_t)

            bias_sb = {}
            for nm, t in bias_rows.items():
                bt = cpool.tile([1, t.shape[1]], BF16, tag=f"bias_{nm}")
                nc.sync.dma_start(bt, t[:])
                bias_sb[nm] = bt

            def proj_feature_major(pp, w_sb, act, out_cb, n_in, n_tok,
                                   bias=None, tag="pp256"):
                """out[oc] = sum_it w.T @ act; out_cb(oc, psum)."""
                for oc in range(DT):
                    ps = pp.tile([P, n_tok], F32, tag=tag, bufs=2)
                    for it in range(n_in):
                        nc.tensor.matmul(ps, w_sb[:, it, oc * P:(oc + 1) * P],
                                         act[:, it, :],
                                         start=(it == 0),
                                         stop=(it == n_in - 1 and bias is None))
                    if bias is not None:
                        nc.tensor.matmul(ps, bias[:, oc * P:(oc + 1) * P],
                                         ones_b[:, :n_tok], start=False,
                                         stop=True)
                    out_cb(oc, ps)

            x_ownT = pers.tile([P, DT, TO], F32R)      # residual stream (own)
            for dt in range(DT):
                nc.sync.dma_start(x_ownT[:, dt, :], xoT_v[dt])

            # cross-attn K2/V2 depend only on the context: computed early in
            # phase B so they overlap everything up to phase E.
            K2_sb = pers.tile([P, DT, CN], BF16)
            V2_sb = pers.tile([P, H, 65], BF16)

            # ========== attn1 scope: phases A-D ==========
            with tc.tile_pool(name="c1", bufs=1) as c1:
                O_sb = c1.tile([P, DT, TO], BF16)
                K_sb = c1.tile([P, DT, N], BF16)
                V_sb = c1.tile([P, NKT, H, 65], BF16)
                Q_sb = c1.tile([P, DT, TO], BF16)
                lnoT = c1.tile([P, DT, TO], BF16)

                with tc.tile_pool(name="c2", bufs=1) as c2:
                    ln1T = c2.tile([P, DT, N], BF16)

                    # ----- Phase A: LN1(own) -> Q1 -> LN1(all tokens) -----
                    scopeA = nc.enter_named_scope("phA_ln1", False)
                    LCH = 512
                    with tc.tile_pool(name="lnps", bufs=2, space="PSUM") as lnp, \
                         tc.tile_pool(name="lnsb", bufs=2) as lnsb:
                        def load_xo(dt, tci, _c={}):
                            if dt not in _c:
                                t = lnsb.tile([P, TO], F32R, tag="xot", bufs=9)
                                nc.sync.dma_start(t, xoT_v[dt])
                                _c[dt] = t
                            return _c[dt]

                        _ln_feature_major(
                            nc, lnp, lnsb, consts, load_xo,
                            lambda dt, tci: lnoT[:, dt, :],
                            DT, TO, TO)

                        def load_x(dt, tci, _c={}):
                            if (dt, tci) not in _c:
                                t = lnsb.tile([P, LCH], F32R, tag="xt", bufs=9)
                                nc.sync.dma_start(
                                    t, xT_v[dt, :, tci * LCH:(tci + 1) * LCH])
                                _c[(dt, tci)] = t
                            return _c[(dt, tci)]

                        _ln_feature_major(
                            nc, lnp, lnsb, consts, load_x,
                            lambda dt, tci: ln1T[:, dt, tci * LCH:(tci + 1) * LCH],
                            DT, N, LCH)
                    nc.leave_named_scope("phA_ln1", scopeA[0], False)

                    # ----- Phase B: K1 / V1 / Q1 (+ K2/V2) projections -----
                    scopeB = nc.enter_named_scope("phB_qkv", False)
                    with tc.tile_pool(name="wb", bufs=1) as wpool, \
                         tc.tile_pool(name="projps", bufs=2, space="PSUM") as pp:
                        # Q1 first: inputs (lnoT, wq1) are ready earliest
                        wq1_sb = wmain.tile([P, DT, D], BF16, tag="w2m", bufs=2)
                        nc.sync.dma_start(wq1_sb, wview(wq1T))
                        proj_feature_major(
                            pp, wq1_sb, lnoT,
                            lambda oc, ps: nc.scalar.copy(out=Q_sb[:, oc, :],
                                                          in_=ps),
                            DT, TO, bias=bias_sb.get("bq1"))

                        # K1: all tokens
                        wk1_sb = wmain.tile([P, DT, D], BF16, tag="w2m", bufs=2)
                        nc.sync.dma_start(wk1_sb, wview(wk1T))
                        bk1 = bias_sb.get("bk1")
                        for oc in range(DT):
                            for tc4 in range(N // 512):
                                k_ps = pp.tile([P, 512], F32, tag="pp512", bufs=3)
                                for it in range(DT):
                                    nc.tensor.matmul(
                                        k_ps, wk1_sb[:, it, oc * P:(oc + 1) * P],
                                        ln1T[:, it, tc4 * 512:(tc4 + 1) * 512],
                                        start=(it == 0),
                                        stop=(it == DT - 1 and bk1 is None))
                                if bk1 is not None:
                                    nc.tensor.matmul(
                                        k_ps, bk1[:, oc * P:(oc + 1) * P],
                                        ones_b, start=False, stop=True)
                                nc.vector.tensor_copy(
                                    out=K_sb[:, oc, tc4 * 512:(tc4 + 1) * 512],
                                    in_=k_ps)

                        # V1: token-major [keys, o] with ones column (col 64)
                        nc.vector.memset(V_sb, 1.0)
                        wv1_sb = wmain.tile([P, DT, D], BF16, tag="w2m", bufs=2)
                        nc.sync.dma_start(wv1_sb, wview(wv1T))
                        bv1 = bias_sb.get("bv1")
                        for kt in range(NKT):
                            for hc in range(2):
                                v_ps = pp.tile([P, 512], F32, tag="pp512", bufs=3)
                                for it in range(DT):
                                    nc.tensor.matmul(
                                        v_ps, ln1T[:, it, kt * P:(kt + 1) * P],
                                        wv1_sb[:, it, hc * 512:(hc + 1) * 512],
                                        start=(it == 0),
                                        stop=(it == DT - 1 and bv1 is None))
                                if bv1 is not None:
                                    nc.tensor.matmul(
                                        v_ps, ones_row.bitcast(BF16),
                                        bv1[:, hc * 512:(hc + 1) * 512],
                                        start=False, stop=True)
                                nc.scalar.copy(
                                    out=V_sb[:, kt, hc * 8:(hc + 1) * 8, 0:64],
                                    in_=v_ps.rearrange("p (h d) -> p h d", d=64))

                        # K2/V2 from context (independent of x)
                        ctx_sb = wpool.tile([P, CT, CN], BF16, tag="ctx", bufs=1)
                        for ct in range(CT):
                            nc.sync.dma_start(ctx_sb[:, ct, :], ctxT_v[ct])
                        wk2_sb = wpool.tile([P, CT, D], BF16, tag="w15", bufs=2)
                        nc.sync.dma_start(wk2_sb, wview(wk2T))
                        for oc in range(DT):
                            k_ps = pp.tile([P, CN], F32, tag="ppsm", bufs=2)
                            for it in range(CT):
                                nc.tensor.matmul(
                                    k_ps, wk2_sb[:, it, oc * P:(oc + 1) * P],
                                    ctx_sb[:, it, :],
                                    start=(it == 0), stop=(it == CT - 1))
                            nc.scalar.copy(out=K2_sb[:, oc, :], in_=k_ps)
                        nc.vector.memset(V2_sb, 1.0)
                        wv2_sb = wpool.tile([P, CT, D], BF16, tag="w15", bufs=2)
                        nc.sync.dma_start(wv2_sb, wview(wv2T))
                        for hc in range(2):
                            v_ps = pp.tile([CN, 512], F32, tag="ppsm", bufs=2)
                            for it in range(CT):
                                nc.tensor.matmul(
                                    v_ps, ctx_sb[:, it, :],
                                    wv2_sb[:, it, hc * 512:(hc + 1) * 512],
                                    start=(it == 0), stop=(it == CT - 1))
                            nc.scalar.copy(
                                out=V2_sb[0:CN, hc * 8:(hc + 1) * 8, 0:64],
                                in_=v_ps.rearrange("p (h d) -> p h d", d=64))
                    nc.leave_named_scope("phB_qkv", scopeB[0], False)

                # ----- Phase C: self-attention heads -----
                scopeC = nc.enter_named_scope("phC_attn", False)
                with tc.tile_pool(name="aps", bufs=1, space="PSUM") as apsum, \
                     tc.tile_pool(name="asb", bufs=1) as asb:
                    o_full = asb.tile([65, H, TO], F32)
                    sums = asb.tile([H, TO], F32)
                    for h in range(H):
                        j, r0 = h >> 1, (h & 1) * 64
                        o_ps = apsum.tile([65, TO], F32, tag="o_ps", bufs=2)
                        for kt in range(NKT):
                            s_ps = apsum.tile([P, TO], F32, tag="s_ps", bufs=3)
                            nc.tensor.matmul(
                                s_ps,
                                K_sb[r0:r0 + 64, j, kt * P:(kt + 1) * P],
                                Q_sb[r0:r0 + 64, j, :],
                                start=True, stop=True)
                            e_t = asb.tile([P, TO], BF16, tag="e_t", bufs=6)
                            nc.scalar.activation(e_t, s_ps, AF.Exp, scale=SCALE)
                            nc.tensor.matmul(
                                o_ps, V_sb[:, kt, h, :], e_t,
                                start=(kt == 0), stop=(kt == NKT - 1))
                        nc.scalar.copy(out=o_full[:, h, :], in_=o_ps)
                        nc.sync.dma_start(sums[h:h + 1, :], o_full[64:65, h, :])
                    r_all = asb.tile([H, TO], F32R)
                    with nc.allow_low_precision("f32r == f32 bits"):
                        nc.vector.reciprocal(r_all, sums)
                    for h in range(H):
                        j, r0 = h >> 1, (h & 1) * 64
                        r_one = asb.tile([1, TO], F32R, tag="r_one", bufs=4)
                        nc.sync.dma_start(r_one, r_all[h:h + 1, :])
                        r_ps = apsum.tile([64, TO], F32, tag="r_ps", bufs=2)
                        nc.tensor.matmul(r_ps, ones_row[:, :64], r_one,
                                         start=True, stop=True)
                        o_tmp = asb.tile([64, TO], BF16, tag="o_tmp", bufs=4)
                        nc.vector.tensor_tensor(out=o_tmp,
                                                in0=o_full[0:64, h, :],
                                                in1=r_ps, op=OP.mult)
                        nc.sync.dma_start(O_sb[r0:r0 + 64, j, :], o_tmp)
                nc.leave_named_scope("phC_attn", scopeC[0], False)

                # ----- Phase D: attn1 out-proj + residual -----
                scopeD = nc.enter_named_scope("phD_oproj", False)
                with tc.tile_pool(name="dps", bufs=3, space="PSUM") as pp:
                    wo1_sb = wmain.tile([P, DT, D], BF16, tag="w2m", bufs=2)
                    nc.sync.dma_start(wo1_sb, wview(wo1T))

                    def add_residual(oc, ps):
                        nc.vector.tensor_tensor(
                            out=x_ownT[:, oc, :],
                            in0=x_ownT[:, oc, :].bitcast(F32),
                            in1=ps, op=OP.add)

                    proj_feature_major(pp, wo1_sb, O_sb, add_residual, DT, TO,
                                       bias=bias_sb.get("bo1"))
                nc.leave_named_scope("phD_oproj", scopeD[0], False)

            # ========== attn2 scope: phase E ==========
            scopeE = nc.enter_named_scope("phE_xattn", False)
            with tc.tile_pool(name="ce", bufs=1) as ce:
                ln2T = ce.tile([P, DT, TO], BF16)
                Q2_sb = ce.tile([P, DT, TO], BF16)
                O2_sb = ce.tile([P, DT, TO], BF16)

                with tc.tile_pool(name="lnps2", bufs=2, space="PSUM") as lnp, \
                     tc.tile_pool(name="lnsb2", bufs=2) as lnsb:
                    _ln_feature_major(
                        nc, lnp, lnsb, consts,
                        lambda dt, tci: x_ownT[:, dt, :],
                        lambda dt, tci: ln2T[:, dt, :],
                        DT, TO, TO)

                with tc.tile_pool(name="eps_", bufs=2, space="PSUM") as pp:
                    wq2_sb = wmain.tile([P, DT, D], BF16, tag="w2m", bufs=2)
                    nc.sync.dma_start(wq2_sb, wview(wq2T))
                    proj_feature_major(
                        pp, wq2_sb, ln2T,
                        lambda oc, ps: nc.scalar.copy(out=Q2_sb[:, oc, :],
                                                      in_=ps),
                        DT, TO, bias=bias_sb.get("bq2"))

                with tc.tile_pool(name="aps2", bufs=1, space="PSUM") as apsum, \
                     tc.tile_pool(name="asb2", bufs=1) as asb:
                    o_full2 = asb.tile([65, H, TO], F32)
                    sums2 = asb.tile([H, TO], F32)
                    for h in range(H):
                        j, r0 = h >> 1, (h & 1) * 64
                        s_ps = apsum.tile([CN, TO], F32, tag="s_ps", bufs=3)
                        nc.tensor.matmul(
                            s_ps, K2_sb[r0:r0 + 64, j, :],
                            Q2_sb[r0:r0 + 64, j, :], start=True, stop=True)
                        e_t = asb.tile([CN, TO], BF16, tag="e_t", bufs=4)
                        nc.scalar.activation(e_t, s_ps, AF.Exp, scale=SCALE)
                        o_ps = apsum.tile([65, TO], F32, tag="o_ps", bufs=2)
                        nc.tensor.matmul(o_ps, V2_sb[0:CN, h, :], e_t,
                                         start=True, stop=True)
                        nc.scalar.copy(out=o_full2[:, h, :], in_=o_ps)
                        nc.sync.dma_start(sums2[h:h + 1, :],
                                          o_full2[64:65, h, :])
                    r_all2 = asb.tile([H, TO], F32R)
                    with nc.allow_low_precision("f32r == f32 bits"):
                        nc.vector.reciprocal(r_all2, sums2)
                    for h in range(H):
                        j, r0 = h >> 1, (h & 1) * 64
                        r_one = asb.tile([1, TO], F32R, tag="r_one", bufs=4)
                        nc.sync.dma_start(r_one, r_all2[h:h + 1, :])
                        r_ps = apsum.tile([64, TO], F32, tag="r_ps", bufs=2)
                        nc.tensor.matmul(r_ps, ones_row[:, :64], r_one,
                                         start=True, stop=True)
                        o_tmp = asb.tile([64, TO], BF16, tag="o_tmp", bufs=4)
                        nc.vector.tensor_tensor(out=o_tmp,
                                                in0=o_full2[0:64, h, :],
                                                in1=r_ps, op=OP.mult)
                        nc.sync.dma_start(O2_sb[r0:r0 + 64, j, :], o_tmp)

                with tc.tile_pool(name="eps2", bufs=3, space="PSUM") as pp:
                    wo2_sb = wmain.tile([P, DT, D], BF16, tag="w2m", bufs=2)
                    nc.sync.dma_start(wo2_sb, wview(wo2T))

                    def add_residual2(oc, ps):
                        nc.vector.tensor_tensor(
                            out=x_ownT[:, oc, :],
                            in0=x_ownT[:, oc, :].bitcast(F32),
                            in1=ps, op=OP.add)

                    proj_feature_major(pp, wo2_sb, O2_sb, add_residual2, DT, TO,
                                       bias=bias_sb.get("bo2"))
            nc.leave_named_scope("phE_xattn", scopeE[0], False)

            # ========== FFN scope: phase F ==========
            scopeF = nc.enter_named_scope("phF_ffn", False)
            with tc.tile_pool(name="cf", bufs=1) as cf:
                ln3T = cf.tile([P, DT, TO], BF16)
                Hbuf = cf.tile([P, FT, TO], BF16)

                with tc.tile_pool(name="lnps3", bufs=2, space="PSUM") as lnp, \
                     tc.tile_pool(name="lnsb3", bufs=2) as lnsb:
                    _ln_feature_major(
                        nc, lnp, lnsb, consts,
                        lambda dt, tci: x_ownT[:, dt, :],
                        lambda dt, tci: ln3T[:, dt, :],
                        DT, TO, TO)

                wgT_v = wview(wgT)
                bgeg = bias_sb.get("bgeg")
                with tc.tile_pool(name="wg", bufs=1) as wgpool, \
                     tc.tile_pool(name="gps", bufs=1, space="PSUM") as gpsum, \
                     tc.tile_pool(name="gsb", bufs=3) as gsb:
                    for g in range(8):
                        wg_h = wgpool.tile([P, DT, 512], BF16, tag="wgh", bufs=2)
                        nc.sync.dma_start(wg_h,
                                          wgT_v[:, :, g * 512:(g + 1) * 512])
                        wg_g = wgpool.tile([P, DT, 512], BF16, tag="wgg", bufs=2)
                        nc.sync.dma_start(
                            wg_g, wgT_v[:, :, FF + g * 512:FF + (g + 1) * 512])
                        for fi in range(4):
                            f = g * 4 + fi
                            h_ps = gpsum.tile([P, TO], F32, tag="h_ps", bufs=2)
                            for it in range(DT):
                                nc.tensor.matmul(
                                    h_ps, wg_h[:, it, fi * P:(fi + 1) * P],
                                    ln3T[:, it, :],
                                    start=(it == 0),
                                    stop=(it == DT - 1 and bgeg is None))
                            if bgeg is not None:
                                nc.tensor.matmul(
                                    h_ps, bgeg[:, f * P:(f + 1) * P],
                                    ones_b[:, :TO], start=False, stop=True)
                            g_ps = gpsum.tile([P, TO], F32, tag="g_ps", bufs=2)
                            for it in range(DT):
                                nc.tensor.matmul(
                                    g_ps, wg_g[:, it, fi * P:(fi + 1) * P],
                                    ln3T[:, it, :],
                                    start=(it == 0),
                                    stop=(it == DT - 1 and bgeg is None))
                            if bgeg is not None:
                                nc.tensor.matmul(
                                    g_ps,
                                    bgeg[:, FF + f * P:FF + (f + 1) * P],
                                    ones_b[:, :TO], start=False, stop=True)
                            gel = gsb.tile([P, TO], F32, tag="gel", bufs=3)
                            nc.scalar.activation(gel, g_ps, AF.Gelu)
                            nc.vector.tensor_tensor(out=Hbuf[:, f, :],
                                                    in0=h_ps, in1=gel,
                                                    op=OP.mult)

                # ffout: two-level accumulation; spills add into x_ownT
                wfT_v = wfT.rearrange("(f p) o -> f p o", p=P)
                bff = bias_sb.get("bff")
                with tc.tile_pool(name="wfp", bufs=1) as wfpool, \
                     tc.tile_pool(name="yps", bufs=2, space="PSUM") as yp_:
                    for fg in range(4):
                        wf_tiles = []
                        for f8 in range(8):
                            wt = wfpool.tile([P, D], BF16, tag="wft", bufs=10)
                            nc.sync.dma_start(wt, wfT_v[fg * 8 + f8])
                            wf_tiles.append(wt)
                        for oc in range(DT):
                            i_ps = yp_.tile([P, TO], F32, tag="i_ps")
                            add_bias = bff is not None and fg == 3
                            for f8 in range(8):
                                nc.tensor.matmul(
                                    i_ps, wf_tiles[f8][:, oc * P:(oc + 1) * P],
                                    Hbuf[:, fg * 8 + f8, :],
                                    start=(f8 == 0),
                                    stop=(f8 == 7 and not add_bias))
                            if add_bias:
                                nc.tensor.matmul(
                                    i_ps, bff[:, oc * P:(oc + 1) * P],
                                    ones_b[:, :TO], start=False, stop=True)
                            nc.vector.tensor_tensor(
                                out=x_ownT[:, oc, :],
                                in0=x_ownT[:, oc, :].bitcast(F32),
                                in1=i_ps, op=OP.add)
            nc.leave_named_scope("phF_ffn", scopeF[0], False)

            nc.sync.dma_start(yT_v, x_ownT)

    nc.finalize()
    return nc


_CACHE = {}


def kernel(**inputs):
    def f32c(a):
        return np.ascontiguousarray(np.asarray(a, dtype=np.float32))

    def bfT(w):
        """W [out,in] (optionally gain-folded) -> bf16 W.T contiguous."""
        return np.ascontiguousarray(w.T).astype(ml_dtypes.bfloat16)

    x = f32c(inputs["hidden_states"])[0]          # [N, D]
    ctx = f32c(inputs["context"])[0]              # [CN, CD]
    g1 = f32c(inputs["ln1_g"]); b1 = f32c(inputs["ln1_b"])
    g2 = f32c(inputs["ln2_g"]); b2 = f32c(inputs["ln2_b"])
    g3 = f32c(inputs["ln3_g"]); b3 = f32c(inputs["ln3_b"])
    wq1 = f32c(inputs["wq1"]); wk1 = f32c(inputs["wk1"]); wv1 = f32c(inputs["wv1"])
    wo1 = f32c(inputs["wo1"]); bo1 = f32c(inputs["bo1"])
    wq2 = f32c(inputs["wq2"]); wk2 = f32c(inputs["wk2"]); wv2 = f32c(inputs["wv2"])
    wo2 = f32c(inputs["wo2"]); bo2 = f32c(inputs["bo2"])
    wg = f32c(inputs["w_geglu"]); bg = f32c(inputs["b_geglu"])
    wf = f32c(inputs["w_ffout"]); bf = f32c(inputs["b_ffout"])

    bq1 = wq1 @ b1; bk1 = wk1 @ b1; bv1 = wv1 @ b1
    bq2 = wq2 @ b2
    bgeg = bg + wg @ b3
    flags = (bool(np.any(bq1) or np.any(bk1) or np.any(bv1)), bool(np.any(bo1)),
             bool(np.any(bq2)), bool(np.any(bo2)), bool(np.any(bgeg)),
             bool(np.any(bf)))

    if flags not in _CACHE:
        _CACHE[flags] = build(flags)
    nc = _CACHE[flags]

    xT = np.ascontiguousarray(x.T)                # [D, N]
    bf16 = ml_dtypes.bfloat16
    shared = {
        "xT": xT,
        "ctxT": np.ascontiguousarray(ctx.T).astype(bf16),
        "wq1T": bfT(wq1 * g1[None, :]),
        "wk1T": bfT(wk1 * g1[None, :]),
        "wv1T": bfT(wv1 * g1[None, :]),
        "wo1T": bfT(wo1),
        "wq2T": bfT(wq2 * g2[None, :]),
        "wk2T": bfT(wk2),
        "wv2T": bfT(wv2),
        "wo2T": bfT(wo2),
        "wgT": bfT(wg * g3[None, :]),
        "wfT": bfT(wf),
        "onesc": np.ones((P, 1), np.float32),
        "onesr": np.ones((1, P), np.float32),
        "onesb": np.ones((1, 512), bf16),
    }
    if flags[0]:
        shared["bq1"] = bq1[None, :].astype(bf16)
        shared["bk1"] = bk1[None, :].astype(bf16)
        shared["bv1"] = bv1[None, :].astype(bf16)
    if flags[1]:
        shared["bo1"] = bo1[None, :].astype(bf16)
    if flags[2]:
        shared["bq2"] = bq2[None, :].astype(bf16)
    if flags[3]:
        shared["bo2"] = bo2[None, :].astype(bf16)
    if flags[4]:
        shared["bgeg"] = bgeg[None, :].astype(bf16)
    if flags[5]:
        shared["bff"] = bf[None, :].astype(bf16)

    in_maps = []
    for c in range(NCORES):
        m = dict(shared)
        m["xoT"] = np.ascontiguousarray(xT[:, c * TO:(c + 1) * TO])
        in_maps.append(m)

    res = run_bass_kernel_spmd(nc, in_maps, core_ids=list(range(NCORES)))
    yT = np.concatenate([r["yT"] for r in res.results], axis=1)  # [D, N]
    return np.ascontiguousarray(yT.T)[None].astype(np.float32)


# revision 16
# speedup vs baseline: 1.0730x; 1.0730x over previous
"""Trainium2 Bass kernel for nn_BasicTransformerBlock (self-attn + cross-attn + GEGLU).

Sharding: data-parallel over the 2048 tokens (256 per core, 8 cores, no
collectives). K/V for self-attention are computed replicated on every core.

On-chip layout is feature-major throughout ([feature(part), token(free)]).
Host pre-packs weights as bf16 W.T (C-contiguous [in, out]) and pre-transposes
x / context, so the device does zero transposes/casts and all DMAs are
contiguous. Weight/projection matmuls run in bf16 (fp32 PSUM accumulate);
LayerNorm statistics run in float32r off the fp32 residual stream.

Softmax: scores are computed keys-on-partitions ([keys, q]); exp on ScalarE
with the 1/8 scale folded in; denominators come from an appended ones-column
in V (row 64 of the AV accumulation); all 16 head sums are staged into one
tile, inverted with a single reciprocal, and divided in via a PE-broadcast.
"""

import numpy as np
import ml_dtypes

import concourse.bass as bass
import concourse.mybir as mybir
import concourse.tile as tile
from concourse import bacc
from concourse.bass_utils import run_bass_kernel_spmd

F32 = mybir.dt.float32
F32R = mybir.dt.float32r
BF16 = mybir.dt.bfloat16
AF = mybir.ActivationFunctionType
OP = mybir.AluOpType

P = 128
N, D = 2048, 1024
H, DH = 16, 64
CN, CD = 77, 768
FF = 4096
EPS = 1e-5
SCALE = DH ** -0.5
NCORES = 8
TO = N // NCORES          # 256 tokens owned per core
DT = D // P               # 8 feature tiles
CT = CD // P              # 6 context-feature tiles
NKT = N // P              # 16 key tiles
FT = FF // P              # 32 ffn-inner tiles


def _ln_feature_major(nc, lnp, sbp, consts, src_of, dst_of, n_dt, tn, chunk):
    """Un-affine LayerNorm over feature-major f32r data."""
    ones_col, ones_row, eps_t = consts
    inv_d = 1.0 / (n_dt * P)
    for tci in range(tn // chunk):
        srcs = [src_of(dt, tci) for dt in range(n_dt)]   # f32r tiles
        sum_ps = lnp.tile([1, chunk], F32, tag="ln_sum", bufs=2)
        for dt in range(n_dt):
            nc.tensor.matmul(sum_ps, ones_col, srcs[dt],
                             start=(dt == 0), stop=(dt == n_dt - 1))
        sumsq_ps = lnp.tile([1, chunk], F32, tag="ln_sumsq", bufs=2)
        for dt in range(n_dt):
            sq_t = sbp.tile([P, chunk], F32R, tag="ln_sq", bufs=2)
            nc.scalar.activation(sq_t, srcs[dt].bitcast(F32), AF.Square)
            nc.tensor.matmul(sumsq_ps, ones_col, sq_t,
                             start=(dt == 0), stop=(dt == n_dt - 1))
        mu_row = sbp.tile([1, chunk], F32R, tag="ln_mu", bufs=2)
        nc.scalar.mul(out=mu_row, in_=sum_ps, mul=inv_d)
        var_row = sbp.tile([1, chunk], F32, tag="ln_var", bufs=2)
        nc.scalar.mul(out=var_row, in_=sumsq_ps, mul=inv_d)
        musq = sbp.tile([1, chunk], F32, tag="ln_musq", bufs=2)
        nc.vector.tensor_mul(out=musq, in0=mu_row.bitcast(F32),
                             in1=mu_row.bitcast(F32))
        nc.vector.tensor_tensor(out=var_row, in0=var_row, in1=musq,
                                op=OP.subtract)
        nc.scalar.activation(var_row, var_row, AF.Sqrt, bias=eps_t)
        rstd_row = sbp.tile([1, chunk], F32R, tag="ln_rstd", bufs=2)
        with nc.allow_low_precision("f32r keeps full fp32 bits here"):
            nc.vector.reciprocal(rstd_row, var_row)
        mu_b = lnp.tile([P, chunk], F32, tag="ln_mub", bufs=2)
        nc.tensor.matmul(mu_b, ones_row, mu_row, start=True, stop=True)
        rstd_b = lnp.tile([P, chunk], F32, tag="ln_rstdb", bufs=2)
        nc.tensor.matmul(rstd_b, ones_row, rstd_row, start=True, stop=True)
        for dt in range(n_dt):
            tmp = sbp.tile([P, chunk], F32, tag="ln_tmp", bufs=2)
            nc.vector.tensor_tensor(out=tmp, in0=srcs[dt].bitcast(F32),
                                    in1=mu_b, op=OP.subtract)
            nc.vector.tensor_tensor(out=dst_of(dt, tci), in0=tmp, in1=rstd_b,
                                    op=OP.mult)


def build(flags):
    has_qkv1b, has_bo1, has_q2b, has_bo2, has_gegb, has_ffb = flags
    nc = bacc.Bacc()

    xT = nc.dram_tensor("xT", [D, N], F32R, kind="ExternalInput")
    xoT = nc.dram_tensor("xoT", [D, TO], F32R, kind="ExternalInput")
    ctxT = nc.dram_tensor("ctxT", [CD, CN], BF16, kind="ExternalInput")
    wq1T = nc.dram_tensor("wq1T", [D, D], BF16, kind="ExternalInput")
    wk1T = nc.dram_tensor("wk1T", [D, D], BF16, kind="ExternalInput")
    wv1T = nc.dram_tensor("wv1T", [D, D], BF16, kind="ExternalInput")
    wo1T = nc.dram_tensor("wo1T", [D, D], BF16, kind="ExternalInput")
    wq2T = nc.dram_tensor("wq2T", [D, D], BF16, kind="ExternalInput")
    wk2T = nc.dram_tensor("wk2T", [CD, D], BF16, kind="ExternalInput")
    wv2T = nc.dram_tensor("wv2T", [CD, D], BF16, kind="ExternalInput")
    wo2T = nc.dram_tensor("wo2T", [D, D], BF16, kind="ExternalInput")
    wgT = nc.dram_tensor("wgT", [D, 2 * FF], BF16, kind="ExternalInput")
    wfT = nc.dram_tensor("wfT", [FF, D], BF16, kind="ExternalInput")
    onesc = nc.dram_tensor("onesc", [P, 1], F32R, kind="ExternalInput")
    onesr = nc.dram_tensor("onesr", [1, P], F32R, kind="ExternalInput")
    onesb = nc.dram_tensor("onesb", [1, 512], BF16, kind="ExternalInput")
    bias_rows = {}
    if has_qkv1b:
        for nm in ("bq1", "bk1", "bv1"):
            bias_rows[nm] = nc.dram_tensor(nm, [1, D], BF16, kind="ExternalInput")
    if has_bo1:
        bias_rows["bo1"] = nc.dram_tensor("bo1", [1, D], BF16, kind="ExternalInput")
    if has_q2b:
        bias_rows["bq2"] = nc.dram_tensor("bq2", [1, D], BF16, kind="ExternalInput")
    if has_bo2:
        bias_rows["bo2"] = nc.dram_tensor("bo2", [1, D], BF16, kind="ExternalInput")
    if has_gegb:
        bias_rows["bgeg"] = nc.dram_tensor("bgeg", [1, 2 * FF], BF16,
                                           kind="ExternalInput")
    if has_ffb:
        bias_rows["bff"] = nc.dram_tensor("bff", [1, D], BF16, kind="ExternalInput")
    yT = nc.dram_tensor("yT", [D, TO], F32R, kind="ExternalOutput")

    xT_v = xT.rearrange("(dt p) t -> dt p t", p=P)
    xoT_v = xoT.rearrange("(dt p) t -> dt p t", p=P)
    ctxT_v = ctxT.rearrange("(ct p) t -> ct p t", p=P)
    yT_v = yT.rearrange("(dt p) t -> p dt t", p=P)

    def wview(w):
        return w.rearrange("(it p) o -> p it o", p=P)

    with tile.TileContext(nc) as tc:
        with tc.tile_pool(name="consts", bufs=1) as cpool, \
             tc.tile_pool(name="pers", bufs=1) as pers, \
             tc.tile_pool(name="wmain", bufs=1) as wmain:

            ones_col = cpool.tile([P, 1], F32R)
            nc.sync.dma_start(ones_col, onesc[:])
            ones_row = cpool.tile([1, P], F32R)
            nc.sync.dma_start(ones_row, onesr[:])
            ones_b = cpool.tile([1, 512], BF16)
            nc.sync.dma_start(ones_b, onesb[:])
            eps_t = cpool.tile([1, 1], F32)
            nc.vector.memset(eps_t, EPS)
            consts = (ones_col, ones_row, eps_t)

            bias_sb = {}
            for nm, t in bias_rows.items():
                bt = cpool.tile([1, t.shape[1]], BF16, tag=f"bias_{nm}")
                nc.sync.dma_start(bt, t[:])
                bias_sb[nm] = bt

            def proj_feature_major(pp, w_sb, act, out_cb, n_in, n_tok,
                                   bias=None, tag="pp256"):
                """out[oc] = sum_it w.T @ act; out_cb(oc, psum)."""
                for oc in range(DT):
                    ps = pp.tile([P, n_tok], F32, tag=tag, bufs=2)
                    for it in range(n_in):
                        nc.tensor.matmul(ps, w_sb[:, it, oc * P:(oc + 1) * P],
                                         act[:, it, :],
                                         start=(it == 0),
                                         stop=(it == n_in - 1 and bias is None))
                    if bias is not None:
                        nc.tensor.matmul(ps, bias[:, oc * P:(oc + 1) * P],
                                         ones_b[:, :n_tok], start=False,
                                         stop=True)
                    out_cb(oc, ps)

            x_ownT = pers.tile([P, DT, TO], F32R)      # residual stream (own)
            for dt in range(DT):
                nc.sync.dma_start(x_ownT[:, dt, :], xoT_v[dt])

            # cross-attn K2/V2 depend only on the context: computed early in
            # phase B so they overlap everything up to phase E.
            K2_sb = pers.tile([P, DT, CN], BF16)
            V2_sb = pers.tile([P, H, 65], BF16)

            # ========== attn1 scope: phases A-D ==========
            with tc.tile_pool(name="c1", bufs=1) as c1:
                O_sb = c1.tile([P, DT, TO], BF16)
                K_sb = c1.tile([P, DT, N], BF16)
                V_sb = c1.tile([P, NKT, H, 65], BF16)
                Q_sb = c1.tile([P, DT, TO], BF16)
                lnoT = c1.tile([P, DT, TO], BF16)

                with tc.tile_pool(name="c2", bufs=1) as c2:
                    ln1T = c2.tile([P, DT, N], BF16)

                    # ----- Phase A: LN1(own) -> Q1 -> LN1(all tokens) -----
                    scopeA = nc.enter_named_scope("phA_ln1", False)
                    LCH = 512
                    with tc.tile_pool(name="lnps", bufs=2, space="PSUM") as lnp, \
                         tc.tile_pool(name="lnsb", bufs=2) as lnsb:
                        _ln_feature_major(
                            nc, lnp, lnsb, consts,
                            lambda dt, tci: x_ownT[:, dt, :],
                            lambda dt, tci: lnoT[:, dt, :],
                            DT, TO, TO)

                        def load_x(dt, tci, _c={}):
                            if (dt, tci) not in _c:
                                t = lnsb.tile([P, LCH], F32R, tag="xt", bufs=9)
                                nc.sync.dma_start(
                                    t, xT_v[dt, :, tci * LCH:(tci + 1) * LCH])
                                _c[(dt, tci)] = t
                            return _c[(dt, tci)]

                        _ln_feature_major(
                            nc, lnp, lnsb, consts, load_x,
                            lambda dt, tci: ln1T[:, dt, tci * LCH:(tci + 1) * LCH],
                            DT, N, LCH)
                    nc.leave_named_scope("phA_ln1", scopeA[0], False)

                    # ----- Phase B: K1 / V1 / Q1 (+ K2/V2) projections -----
                    scopeB = nc.enter_named_scope("phB_qkv", False)
                    with tc.tile_pool(name="wb", bufs=1) as wpool, \
                         tc.tile_pool(name="projps", bufs=2, space="PSUM") as pp:
                        # Q1 first: inputs (lnoT, wq1) are ready earliest
                        wq1_sb = wmain.tile([P, DT, D], BF16, tag="w2m", bufs=2)
                        nc.sync.dma_start(wq1_sb, wview(wq1T))
                        proj_feature_major(
                            pp, wq1_sb, lnoT,
                            lambda oc, ps: nc.scalar.copy(out=Q_sb[:, oc, :],
                                                          in_=ps),
                            DT, TO, bias=bias_sb.get("bq1"))

                        # K1: all tokens
                        wk1_sb = wmain.tile([P, DT, D], BF16, tag="w2m", bufs=2)
                        nc.sync.dma_start(wk1_sb, wview(wk1T))
                        bk1 = bias_sb.get("bk1")
                        for oc in range(DT):
                            for tc4 in range(N // 512):
                                k_ps = pp.tile([P, 512], F32, tag="pp512", bufs=3)
                                for it in range(DT):
                                    nc.tensor.matmul(
                                        k_ps, wk1_sb[:, it, oc * P:(oc + 1) * P],
                                        ln1T[:, it, tc4 * 512:(tc4 + 1) * 512],
                                        start=(it == 0),
                                        stop=(it == DT - 1 and bk1 is None))
                                if bk1 is not None:
                                    nc.tensor.matmul(
                                        k_ps, bk1[:, oc * P:(oc + 1) * P],
                                        ones_b, start=False, stop=True)
                                nc.vector.tensor_copy(
                                    out=K_sb[:, oc, tc4 * 512:(tc4 + 1) * 512],
                                    in_=k_ps)

                        # V1: token-major [keys, o] with ones column (col 64)
                        nc.vector.memset(V_sb, 1.0)
                        wv1_sb = wmain.tile([P, DT, D], BF16, tag="w2m", bufs=2)
                        nc.sync.dma_start(wv1_sb, wview(wv1T))
                        bv1 = bias_sb.get("bv1")
                        for kt in range(NKT):
                            for hc in range(2):
                                v_ps = pp.tile([P, 512], F32, tag="pp512", bufs=3)
                                for it in range(DT):
                                    nc.tensor.matmul(
                                        v_ps, ln1T[:, it, kt * P:(kt + 1) * P],
                                        wv1_sb[:, it, hc * 512:(hc + 1) * 512],
                                        start=(it == 0),
                                        stop=(it == DT - 1 and bv1 is None))
                                if bv1 is not None:
                                    nc.tensor.matmul(
                                        v_ps, ones_row.bitcast(BF16),
                                        bv1[:, hc * 512:(hc + 1) * 512],
                                        start=False, stop=True)
                                nc.scalar.copy(
                                    out=V_sb[:, kt, hc * 8:(hc + 1) * 8, 0:64],
                                    in_=v_ps.rearrange("p (h d) -> p h d", d=64))

                        # K2/V2 from context (independent of x)
                        ctx_sb = wpool.tile([P, CT, CN], BF16, tag="ctx", bufs=1)
                        for ct in range(CT):
                            nc.sync.dma_start(ctx_sb[:, ct, :], ctxT_v[ct])
                        wk2_sb = wpool.tile([P, CT, D], BF16, tag="w15", bufs=2)
                        nc.sync.dma_start(wk2_sb, wview(wk2T))
                        for oc in range(DT):
                            k_ps = pp.tile([P, CN], F32, tag="ppsm", bufs=2)
                            for it in range(CT):
                                nc.tensor.matmul(
                                    k_ps, wk2_sb[:, it, oc * P:(oc + 1) * P],
                                    ctx_sb[:, it, :],
                                    start=(it == 0), stop=(it == CT - 1))
                            nc.scalar.copy(out=K2_sb[:, oc, :], in_=k_ps)
                        nc.vector.memset(V2_sb, 1.0)
                        wv2_sb = wpool.tile([P, CT, D], BF16, tag="w15", bufs=2)
                        nc.sync.dma_start(wv2_sb, wview(wv2T))
                        for hc in range(2):
                            v_ps = pp.tile([CN, 512], F32, tag="ppsm", bufs=2)
                            for it in range(CT):
                                nc.tensor.matmul(
                                    v_ps, ctx_sb[:, it, :],
                                    wv2_sb[:, it, hc * 512:(hc + 1) * 512],
                                    start=(it == 0), stop=(it == CT - 1))
                            nc.scalar.copy(
                                out=V2_sb[0:CN, hc * 8:(hc + 1) * 8, 0:64],
                                in_=v_ps.rearrange("p (h d) -> p h d", d=64))
                    nc.leave_named_scope("phB_qkv", scopeB[0], False)

                # ----- Phase C: self-attention heads -----
                scopeC = nc.enter_named_scope("phC_attn", False)
                with tc.tile_pool(name="aps", bufs=1, space="PSUM") as apsum, \
                     tc.tile_pool(name="asb", bufs=1) as asb:
                    for h in range(H):
                        j, r0 = h >> 1, (h & 1) * 64
                        o_ps = apsum.tile([65, TO], F32, tag="o_ps", bufs=2)
                        for kt2 in range(NKT // 2):
                            # two key-tiles share one PSUM bank; second matmul
                            # overwrites its (cleared) half via has_written
                            s_ps = apsum.tile([P, 2, TO], F32, tag="s_ps", bufs=3)
                            for half in range(2):
                                kt = kt2 * 2 + half
                                nc.tensor.matmul(
                                    s_ps[:, half, :],
                                    K_sb[r0:r0 + 64, j, kt * P:(kt + 1) * P],
                                    Q_sb[r0:r0 + 64, j, :],
                                    start=(half == 0), stop=True,
                                    skip_group_check=(half == 1))
                            e_t = asb.tile([P, 2, TO], BF16, tag="e_t", bufs=6)
                            nc.scalar.activation(e_t, s_ps, AF.Exp, scale=SCALE)
                            for half in range(2):
                                kt = kt2 * 2 + half
                                nc.tensor.matmul(
                                    o_ps, V_sb[:, kt, h, :], e_t[:, half, :],
                                    start=(kt == 0), stop=(kt == NKT - 1))
                        r_sb = asb.tile([1, TO], F32R, tag="r_sb", bufs=4)
                        with nc.allow_low_precision("f32r == f32 bits"):
                            nc.vector.reciprocal(r_sb, o_ps[64:65, :])
                        r_ps = apsum.tile([64, TO], F32, tag="r_ps", bufs=2)
                        nc.tensor.matmul(r_ps, ones_row[:, :64], r_sb,
                                         start=True, stop=True)
                        r_bc = asb.tile([64, TO], F32, tag="r_bc", bufs=3)
                        nc.scalar.copy(out=r_bc, in_=r_ps)
                        nc.vector.tensor_tensor(out=O_sb[r0:r0 + 64, j, :],
                                                in0=o_ps[0:64, :],
                                                in1=r_bc, op=OP.mult)
                nc.leave_named_scope("phC_attn", scopeC[0], False)

                # ----- Phase D: attn1 out-proj + residual -----
                scopeD = nc.enter_named_scope("phD_oproj", False)
                with tc.tile_pool(name="dps", bufs=3, space="PSUM") as pp:
                    wo1_sb = wmain.tile([P, DT, D], BF16, tag="w2m", bufs=2)
                    nc.sync.dma_start(wo1_sb, wview(wo1T))

                    def add_residual(oc, ps):
                        nc.vector.tensor_tensor(
                            out=x_ownT[:, oc, :],
                            in0=x_ownT[:, oc, :].bitcast(F32),
                            in1=ps, op=OP.add)

                    proj_feature_major(pp, wo1_sb, O_sb, add_residual, DT, TO,
                                       bias=bias_sb.get("bo1"))
                nc.leave_named_scope("phD_oproj", scopeD[0], False)

            # ========== attn2 scope: phase E ==========
            scopeE = nc.enter_named_scope("phE_xattn", False)
            with tc.tile_pool(name="ce", bufs=1) as ce:
                ln2T = ce.tile([P, DT, TO], BF16)
                Q2_sb = ce.tile([P, DT, TO], BF16)
                O2_sb = ce.tile([P, DT, TO], BF16)

                with tc.tile_pool(name="lnps2", bufs=2, space="PSUM") as lnp, \
                     tc.tile_pool(name="lnsb2", bufs=2) as lnsb:
                    _ln_feature_major(
                        nc, lnp, lnsb, consts,
                        lambda dt, tci: x_ownT[:, dt, :],
                        lambda dt, tci: ln2T[:, dt, :],
                        DT, TO, TO)

                with tc.tile_pool(name="eps_", bufs=2, space="PSUM") as pp:
                    wq2_sb = wmain.tile([P, DT, D], BF16, tag="w2m", bufs=2)
                    nc.sync.dma_start(wq2_sb, wview(wq2T))
                    proj_feature_major(
                        pp, wq2_sb, ln2T,
                        lambda oc, ps: nc.scalar.copy(out=Q2_sb[:, oc, :],
                                                      in_=ps),
                        DT, TO, bias=bias_sb.get("bq2"))

                with tc.tile_pool(name="aps2", bufs=1, space="PSUM") as apsum, \
                     tc.tile_pool(name="asb2", bufs=1) as asb:
                    for h in range(H):
                        j, r0 = h >> 1, (h & 1) * 64
                        s_ps = apsum.tile([CN, TO], F32, tag="s_ps", bufs=3)
                        nc.tensor.matmul(
                            s_ps, K2_sb[r0:r0 + 64, j, :],
                            Q2_sb[r0:r0 + 64, j, :], start=True, stop=True)
                        e_t = asb.tile([CN, TO], BF16, tag="e_t", bufs=4)
                        nc.scalar.activation(e_t, s_ps, AF.Exp, scale=SCALE)
                        o_ps = apsum.tile([65, TO], F32, tag="o_ps", bufs=2)
                        nc.tensor.matmul(o_ps, V2_sb[0:CN, h, :], e_t,
                                         start=True, stop=True)
                        r_sb = asb.tile([1, TO], F32R, tag="r_sb", bufs=4)
                        with nc.allow_low_precision("f32r == f32 bits"):
                            nc.vector.reciprocal(r_sb, o_ps[64:65, :])
                        r_ps = apsum.tile([64, TO], F32, tag="r_ps", bufs=2)
                        nc.tensor.matmul(r_ps, ones_row[:, :64], r_sb,
                                         start=True, stop=True)
                        r_bc = asb.tile([64, TO], F32, tag="r_bc", bufs=3)
                        nc.scalar.copy(out=r_bc, in_=r_ps)
                        nc.vector.tensor_tensor(out=O2_sb[r0:r0 + 64, j, :],
                                                in0=o_ps[0:64, :],
                                                in1=r_bc, op=OP.mult)

                with tc.tile_pool(name="eps2", bufs=3, space="PSUM") as pp:
                    wo2_sb = wmain.tile([P, DT, D], BF16, tag="w2m", bufs=2)
                    nc.sync.dma_start(wo2_sb, wview(wo2T))

                    def add_residual2(oc, ps):
                        nc.vector.tensor_tensor(
                            out=x_ownT[:, oc, :],
                            in0=x_ownT[:, oc, :].bitcast(F32),
                            in1=ps, op=OP.add)

                    proj_feature_major(pp, wo2_sb, O2_sb, add_residual2, DT, TO,
                                       bias=bias_sb.get("bo2"))
            nc.leave_named_scope("phE_xattn", scopeE[0], False)

            # ========== FFN scope: phase F ==========
            scopeF = nc.enter_named_scope("phF_ffn", False)
            with tc.tile_pool(name="cf", bufs=1) as cf:
                ln3T = cf.tile([P, DT, TO], BF16)
                Hbuf = cf.tile([P, FT, TO], BF16)

                with tc.tile_pool(name="lnps3", bufs=2, space="PSUM") as lnp, \
                     tc.tile_pool(name="lnsb3", bufs=2) as lnsb:
                    _ln_feature_major(
                        nc, lnp, lnsb, consts,
                        lambda dt, tci: x_ownT[:, dt, :],
                        lambda dt, tci: ln3T[:, dt, :],
                        DT, TO, TO)

                wgT_v = wview(wgT)
                bgeg = bias_sb.get("bgeg")
                with tc.tile_pool(name="wg", bufs=1) as wgpool, \
                     tc.tile_pool(name="gps", bufs=1, space="PSUM") as gpsum, \
                     tc.tile_pool(name="gsb", bufs=3) as gsb:
                    for g in range(8):
                        wg_h = wgpool.tile([P, DT, 512], BF16, tag="wgh", bufs=2)
                        nc.sync.dma_start(wg_h,
                                          wgT_v[:, :, g * 512:(g + 1) * 512])
                        wg_g = wgpool.tile([P, DT, 512], BF16, tag="wgg", bufs=2)
                        nc.sync.dma_start(
                            wg_g, wgT_v[:, :, FF + g * 512:FF + (g + 1) * 512])
                        for fi in range(4):
                            f = g * 4 + fi
                            h_ps = gpsum.tile([P, TO], F32, tag="h_ps", bufs=2)
                            for it in range(DT):
                                nc.tensor.matmul(
                                    h_ps, wg_h[:, it, fi * P:(fi + 1) * P],
                                    ln3T[:, it, :],
                                    start=(it == 0),
                                    stop=(it == DT - 1 and bgeg is None))
                            if bgeg is not None:
                                nc.tensor.matmul(
                                    h_ps, bgeg[:, f * P:(f + 1) * P],
                                    ones_b[:, :TO], start=False, stop=True)
                            g_ps = gpsum.tile([P, TO], F32, tag="g_ps", bufs=2)
                            for it in range(DT):
                                nc.tensor.matmul(
                                    g_ps, wg_g[:, it, fi * P:(fi + 1) * P],
                                    ln3T[:, it, :],
                                    start=(it == 0),
                                    stop=(it == DT - 1 and bgeg is None))
                            if bgeg is not None:
                                nc.tensor.matmul(
                                    g_ps,
                                    bgeg[:, FF + f * P:FF + (f + 1) * P],
                                    ones_b[:, :TO], start=False, stop=True)
                            gel = gsb.tile([P, TO], F32, tag="gel", bufs=3)
                            nc.scalar.activation(gel, g_ps, AF.Gelu)
                            nc.vector.tensor_tensor(out=Hbuf[:, f, :],
                                                    in0=h_ps, in1=gel,
                                                    op=OP.mult)

                # ffout: two-level accumulation; spills add into x_ownT
                wfT_v = wfT.rearrange("(f p) o -> f p o", p=P)
                bff = bias_sb.get("bff")
                with tc.tile_pool(name="wfp", bufs=1) as wfpool, \
                     tc.tile_pool(name="yps", bufs=2, space="PSUM") as yp_:
                    for fg in range(4):
                        wf_tiles = []
                        for f8 in range(8):
                            wt = wfpool.tile([P, D], BF16, tag="wft", bufs=10)
                            nc.sync.dma_start(wt, wfT_v[fg * 8 + f8])
                            wf_tiles.append(wt)
                        for oc in range(DT):
                            i_ps = yp_.tile([P, TO], F32, tag="i_ps")
                            add_bias = bff is not None and fg == 3
                            for f8 in range(8):
                                nc.tensor.matmul(
                                    i_ps, wf_tiles[f8][:, oc * P:(oc + 1) * P],
                                    Hbuf[:, fg * 8 + f8, :],
                                    start=(f8 == 0),
                                    stop=(f8 == 7 and not add_bias))
                            if add_bias:
                                nc.tensor.matmul(
                                    i_ps, bff[:, oc * P:(oc + 1) * P],
                                    ones_b[:, :TO], start=False, stop=True)
                            nc.vector.tensor_tensor(
                                out=x_ownT[:, oc, :],
                                in0=x_ownT[:, oc, :].bitcast(F32),
                                in1=i_ps, op=OP.add)
            nc.leave_named_scope("phF_ffn", scopeF[0], False)

            nc.sync.dma_start(yT_v, x_ownT)

    nc.finalize()
    return nc


_CACHE = {}


def kernel(**inputs):
    def f32c(a):
        return np.ascontiguousarray(np.asarray(a, dtype=np.float32))

    def bfT(w):
        """W [out,in] (optionally gain-folded) -> bf16 W.T contiguous."""
        return np.ascontiguousarray(w.T).astype(ml_dtypes.bfloat16)

    x = f32c(inputs["hidden_states"])[0]          # [N, D]
    ctx = f32c(inputs["context"])[0]              # [CN, CD]
    g1 = f32c(inputs["ln1_g"]); b1 = f32c(inputs["ln1_b"])
    g2 = f32c(inputs["ln2_g"]); b2 = f32c(inputs["ln2_b"])
    g3 = f32c(inputs["ln3_g"]); b3 = f32c(inputs["ln3_b"])
    wq1 = f32c(inputs["wq1"]); wk1 = f32c(inputs["wk1"]); wv1 = f32c(inputs["wv1"])
    wo1 = f32c(inputs["wo1"]); bo1 = f32c(inputs["bo1"])
    wq2 = f32c(inputs["wq2"]); wk2 = f32c(inputs["wk2"]); wv2 = f32c(inputs["wv2"])
    wo2 = f32c(inputs["wo2"]); bo2 = f32c(inputs["bo2"])
    wg = f32c(inputs["w_geglu"]); bg = f32c(inputs["b_geglu"])
    wf = f32c(inputs["w_ffout"]); bf = f32c(inputs["b_ffout"])

    bq1 = wq1 @ b1; bk1 = wk1 @ b1; bv1 = wv1 @ b1
    bq2 = wq2 @ b2
    bgeg = bg + wg @ b3
    flags = (bool(np.any(bq1) or np.any(bk1) or np.any(bv1)), bool(np.any(bo1)),
             bool(np.any(bq2)), bool(np.any(bo2)), bool(np.any(bgeg)),
             bool(np.any(bf)))

    if flags not in _CACHE:
        _CACHE[flags] = build(flags)
    nc = _CACHE[flags]

    xT = np.ascontiguousarray(x.T)                # [D, N]
    bf16 = ml_dtypes.bfloat16
    shared = {
        "xT": xT,
        "ctxT": np.ascontiguousarray(ctx.T).astype(bf16),
        "wq1T": bfT(wq1 * g1[None, :]),
        "wk1T": bfT(wk1 * g1[None, :]),
        "wv1T": bfT(wv1 * g1[None, :]),
        "wo1T": bfT(wo1),
        "wq2T": bfT(wq2 * g2[None, :]),
        "wk2T": bfT(wk2),
        "wv2T": bfT(wv2),
        "wo2T": bfT(wo2),
        "wgT": bfT(wg * g3[None, :]),
        "wfT": bfT(wf),
        "onesc": np.ones((P, 1), np.float32),
        "onesr": np.ones((1, P), np.float32),
        "onesb": np.ones((1, 512), bf16),
    }
    if flags[0]:
        shared["bq1"] = bq1[None, :].astype(bf16)
        shared["bk1"] = bk1[None, :].astype(bf16)
        shared["bv1"] = bv1[None, :].astype(bf16)
    if flags[1]:
        shared["bo1"] = bo1[None, :].astype(bf16)
    if flags[2]:
        shared["bq2"] = bq2[None, :].astype(bf16)
    if flags[3]:
        shared["bo2"] = bo2[None, :].astype(bf16)
    if flags[4]:
        shared["bgeg"] = bgeg[None, :].astype(bf16)
    if flags[5]:
        shared["bff"] = bf[None, :].astype(bf16)

    in_maps = []
    for c in range(NCORES):
        m = dict(shared)
        m["xoT"] = np.ascontiguousarray(xT[:, c * TO:(c + 1) * TO])
        in_maps.append(m)

    res = run_bass_kernel_spmd(nc, in_maps, core_ids=list(range(NCORES)))
    yT = np.concatenate([r["yT"] for r in res.results], axis=1)  # [D, N]
    return np.ascontiguousarray(yT.T)[None].astype(np.float32)


# revision 17
# speedup vs baseline: 1.0846x; 1.0109x over previous
"""Trainium2 Bass kernel for nn_BasicTransformerBlock (self-attn + cross-attn + GEGLU).

Sharding: data-parallel over the 2048 tokens (256 per core, 8 cores, no
collectives). K/V for self-attention are computed replicated on every core.

On-chip layout is feature-major throughout ([feature(part), token(free)]).
Host pre-packs weights as bf16 W.T (C-contiguous [in, out]) and pre-transposes
x / context, so the device does zero transposes/casts and all DMAs are
contiguous. Weight/projection matmuls run in bf16 (fp32 PSUM accumulate);
LayerNorm statistics run in float32r off the fp32 residual stream.

Softmax: scores are computed keys-on-partitions ([keys, q]); exp on ScalarE
with the 1/8 scale folded in; denominators come from an appended ones-column
in V (row 64 of the AV accumulation); all 16 head sums are staged into one
tile, inverted with a single reciprocal, and divided in via a PE-broadcast.
"""

import numpy as np
import ml_dtypes

import concourse.bass as bass
import concourse.mybir as mybir
import concourse.tile as tile
from concourse import bacc
from concourse.bass_utils import run_bass_kernel_spmd

F32 = mybir.dt.float32
F32R = mybir.dt.float32r
BF16 = mybir.dt.bfloat16
AF = mybir.ActivationFunctionType
OP = mybir.AluOpType

P = 128
N, D = 2048, 1024
H, DH = 16, 64
CN, CD = 77, 768
FF = 4096
EPS = 1e-5
SCALE = DH ** -0.5
NCORES = 8
TO = N // NCORES          # 256 tokens owned per core
DT = D // P               # 8 feature tiles
CT = CD // P              # 6 context-feature tiles
NKT = N // P              # 16 key tiles
FT = FF // P              # 32 ffn-inner tiles


def _ln_feature_major(nc, lnp, sbp, consts, src_of, dst_of, n_dt, tn, chunk):
    """Un-affine LayerNorm over feature-major f32r data."""
    ones_col, ones_row, eps_t = consts
    inv_d = 1.0 / (n_dt * P)
    for tci in range(tn // chunk):
        srcs = [src_of(dt, tci) for dt in range(n_dt)]   # f32r tiles
        sum_ps = lnp.tile([1, chunk], F32, tag="ln_sum", bufs=2)
        for dt in range(n_dt):
            nc.tensor.matmul(sum_ps, ones_col, srcs[dt],
                             start=(dt == 0), stop=(dt == n_dt - 1))
        sumsq_ps = lnp.tile([1, chunk], F32, tag="ln_sumsq", bufs=2)
        for dt in range(n_dt):
            sq_t = sbp.tile([P, chunk], F32R, tag="ln_sq", bufs=2)
            nc.scalar.activation(sq_t, srcs[dt].bitcast(F32), AF.Square)
            nc.tensor.matmul(sumsq_ps, ones_col, sq_t,
                             start=(dt == 0), stop=(dt == n_dt - 1))
        mu_row = sbp.tile([1, chunk], F32R, tag="ln_mu", bufs=2)
        nc.scalar.mul(out=mu_row, in_=sum_ps, mul=inv_d)
        var_row = sbp.tile([1, chunk], F32, tag="ln_var", bufs=2)
        nc.scalar.mul(out=var_row, in_=sumsq_ps, mul=inv_d)
        musq = sbp.tile([1, chunk], F32, tag="ln_musq", bufs=2)
        nc.vector.tensor_mul(out=musq, in0=mu_row.bitcast(F32),
                             in1=mu_row.bitcast(F32))
        nc.vector.tensor_tensor(out=var_row, in0=var_row, in1=musq,
                                op=OP.subtract)
        nc.scalar.activation(var_row, var_row, AF.Sqrt, bias=eps_t)
        rstd_row = sbp.tile([1, chunk], F32R, tag="ln_rstd", bufs=2)
        with nc.allow_low_precision("f32r keeps full fp32 bits here"):
            nc.vector.reciprocal(rstd_row, var_row)
        mu_b = lnp.tile([P, chunk], F32, tag="ln_mub", bufs=2)
        nc.tensor.matmul(mu_b, ones_row, mu_row, start=True, stop=True)
        rstd_b = lnp.tile([P, chunk], F32, tag="ln_rstdb", bufs=2)
        nc.tensor.matmul(rstd_b, ones_row, rstd_row, start=True, stop=True)
        mu_s = sbp.tile([P, chunk], F32, tag="ln_mus", bufs=2)
        nc.scalar.copy(out=mu_s, in_=mu_b)
        for dt in range(n_dt):
            tmp = sbp.tile([P, chunk], F32, tag="ln_tmp", bufs=3)
            nc.gpsimd.tensor_tensor(out=tmp, in0=srcs[dt].bitcast(F32),
                                    in1=mu_s, op=OP.subtract)
            nc.vector.tensor_tensor(out=dst_of(dt, tci), in0=tmp, in1=rstd_b,
                                    op=OP.mult)


def build(flags):
    has_qkv1b, has_bo1, has_q2b, has_bo2, has_gegb, has_ffb = flags
    nc = bacc.Bacc()

    xT = nc.dram_tensor("xT", [D, N], F32R, kind="ExternalInput")
    xoT = nc.dram_tensor("xoT", [D, TO], F32R, kind="ExternalInput")
    ctxT = nc.dram_tensor("ctxT", [CD, CN], BF16, kind="ExternalInput")
    wq1T = nc.dram_tensor("wq1T", [D, D], BF16, kind="ExternalInput")
    wk1T = nc.dram_tensor("wk1T", [D, D], BF16, kind="ExternalInput")
    wv1T = nc.dram_tensor("wv1T", [D, D], BF16, kind="ExternalInput")
    wo1T = nc.dram_tensor("wo1T", [D, D], BF16, kind="ExternalInput")
    wq2T = nc.dram_tensor("wq2T", [D, D], BF16, kind="ExternalInput")
    wk2T = nc.dram_tensor("wk2T", [CD, D], BF16, kind="ExternalInput")
    wv2T = nc.dram_tensor("wv2T", [CD, D], BF16, kind="ExternalInput")
    wo2T = nc.dram_tensor("wo2T", [D, D], BF16, kind="ExternalInput")
    wgT = nc.dram_tensor("wgT", [D, 2 * FF], BF16, kind="ExternalInput")
    wfT = nc.dram_tensor("wfT", [FF, D], BF16, kind="ExternalInput")
    onesc = nc.dram_tensor("onesc", [P, 1], F32R, kind="ExternalInput")
    onesr = nc.dram_tensor("onesr", [1, P], F32R, kind="ExternalInput")
    onesb = nc.dram_tensor("onesb", [1, 512], BF16, kind="ExternalInput")
    bias_rows = {}
    if has_qkv1b:
        for nm in ("bq1", "bk1", "bv1"):
            bias_rows[nm] = nc.dram_tensor(nm, [1, D], BF16, kind="ExternalInput")
    if has_bo1:
        bias_rows["bo1"] = nc.dram_tensor("bo1", [1, D], BF16, kind="ExternalInput")
    if has_q2b:
        bias_rows["bq2"] = nc.dram_tensor("bq2", [1, D], BF16, kind="ExternalInput")
    if has_bo2:
        bias_rows["bo2"] = nc.dram_tensor("bo2", [1, D], BF16, kind="ExternalInput")
    if has_gegb:
        bias_rows["bgeg"] = nc.dram_tensor("bgeg", [1, 2 * FF], BF16,
                                           kind="ExternalInput")
    if has_ffb:
        bias_rows["bff"] = nc.dram_tensor("bff", [1, D], BF16, kind="ExternalInput")
    yT = nc.dram_tensor("yT", [D, TO], F32R, kind="ExternalOutput")

    xT_v = xT.rearrange("(dt p) t -> dt p t", p=P)
    xoT_v = xoT.rearrange("(dt p) t -> dt p t", p=P)
    ctxT_v = ctxT.rearrange("(ct p) t -> ct p t", p=P)
    yT_v = yT.rearrange("(dt p) t -> p dt t", p=P)

    def wview(w):
        return w.rearrange("(it p) o -> p it o", p=P)

    with tile.TileContext(nc) as tc:
        with tc.tile_pool(name="consts", bufs=1) as cpool, \
             tc.tile_pool(name="pers", bufs=1) as pers, \
             tc.tile_pool(name="wmain", bufs=1) as wmain:

            ones_col = cpool.tile([P, 1], F32R)
            nc.sync.dma_start(ones_col, onesc[:])
            ones_row = cpool.tile([1, P], F32R)
            nc.sync.dma_start(ones_row, onesr[:])
            ones_b = cpool.tile([1, 512], BF16)
            nc.sync.dma_start(ones_b, onesb[:])
            eps_t = cpool.tile([1, 1], F32)
            nc.vector.memset(eps_t, EPS)
            consts = (ones_col, ones_row, eps_t)

            bias_sb = {}
            for nm, t in bias_rows.items():
                bt = cpool.tile([1, t.shape[1]], BF16, tag=f"bias_{nm}")
                nc.sync.dma_start(bt, t[:])
                bias_sb[nm] = bt

            def proj_feature_major(pp, w_sb, act, out_cb, n_in, n_tok,
                                   bias=None, tag="pp256"):
                """out[oc] = sum_it w.T @ act; out_cb(oc, psum)."""
                for oc in range(DT):
                    ps = pp.tile([P, n_tok], F32, tag=tag, bufs=2)
                    for it in range(n_in):
                        nc.tensor.matmul(ps, w_sb[:, it, oc * P:(oc + 1) * P],
                                         act[:, it, :],
                                         start=(it == 0),
                                         stop=(it == n_in - 1 and bias is None))
                    if bias is not None:
                        nc.tensor.matmul(ps, bias[:, oc * P:(oc + 1) * P],
                                         ones_b[:, :n_tok], start=False,
                                         stop=True)
                    out_cb(oc, ps)

            x_ownT = pers.tile([P, DT, TO], F32R)      # residual stream (own)
            for dt in range(DT):
                nc.sync.dma_start(x_ownT[:, dt, :], xoT_v[dt])

            # cross-attn K2/V2 depend only on the context: computed early in
            # phase B so they overlap everything up to phase E.
            K2_sb = pers.tile([P, DT, CN], BF16)
            V2_sb = pers.tile([P, H, 65], BF16)

            # ========== attn1 scope: phases A-D ==========
            with tc.tile_pool(name="c1", bufs=1) as c1:
                O_sb = c1.tile([P, DT, TO], BF16)
                K_sb = c1.tile([P, DT, N], BF16)
                V_sb = c1.tile([P, NKT, H, 65], BF16)
                Q_sb = c1.tile([P, DT, TO], BF16)
                lnoT = c1.tile([P, DT, TO], BF16)

                with tc.tile_pool(name="c2", bufs=1) as c2:
                    ln1T = c2.tile([P, DT, N], BF16)

                    # ----- Phase A: LN1(own) -> Q1 -> LN1(all tokens) -----
                    scopeA = nc.enter_named_scope("phA_ln1", False)
                    LCH = 512
                    with tc.tile_pool(name="lnps", bufs=2, space="PSUM") as lnp, \
                         tc.tile_pool(name="lnsb", bufs=2) as lnsb:
                        _ln_feature_major(
                            nc, lnp, lnsb, consts,
                            lambda dt, tci: x_ownT[:, dt, :],
                            lambda dt, tci: lnoT[:, dt, :],
                            DT, TO, TO)

                        def load_x(dt, tci, _c={}):
                            if (dt, tci) not in _c:
                                t = lnsb.tile([P, LCH], F32R, tag="xt", bufs=9)
                                nc.sync.dma_start(
                                    t, xT_v[dt, :, tci * LCH:(tci + 1) * LCH])
                                _c[(dt, tci)] = t
                            return _c[(dt, tci)]

                        _ln_feature_major(
                            nc, lnp, lnsb, consts, load_x,
                            lambda dt, tci: ln1T[:, dt, tci * LCH:(tci + 1) * LCH],
                            DT, N, LCH)
                    nc.leave_named_scope("phA_ln1", scopeA[0], False)

                    # ----- Phase B: K1 / V1 / Q1 (+ K2/V2) projections -----
                    scopeB = nc.enter_named_scope("phB_qkv", False)
                    with tc.tile_pool(name="wb", bufs=1) as wpool, \
                         tc.tile_pool(name="projps", bufs=2, space="PSUM") as pp:
                        # Q1 first: inputs (lnoT, wq1) are ready earliest
                        wq1_sb = wmain.tile([P, DT, D], BF16, tag="w2m", bufs=2)
                        nc.sync.dma_start(wq1_sb, wview(wq1T))
                        proj_feature_major(
                            pp, wq1_sb, lnoT,
                            lambda oc, ps: nc.scalar.copy(out=Q_sb[:, oc, :],
                                                          in_=ps),
                            DT, TO, bias=bias_sb.get("bq1"))

                        # K1: all tokens
                        wk1_sb = wmain.tile([P, DT, D], BF16, tag="w2m", bufs=2)
                        nc.sync.dma_start(wk1_sb, wview(wk1T))
                        bk1 = bias_sb.get("bk1")
                        for oc in range(DT):
                            for tc4 in range(N // 512):
                                k_ps = pp.tile([P, 512], F32, tag="pp512", bufs=3)
                                for it in range(DT):
                                    nc.tensor.matmul(
                                        k_ps, wk1_sb[:, it, oc * P:(oc + 1) * P],
                                        ln1T[:, it, tc4 * 512:(tc4 + 1) * 512],
                                        start=(it == 0),
                                        stop=(it == DT - 1 and bk1 is None))
                                if bk1 is not None:
                                    nc.tensor.matmul(
                                        k_ps, bk1[:, oc * P:(oc + 1) * P],
                                        ones_b, start=False, stop=True)
                                nc.vector.tensor_copy(
                                    out=K_sb[:, oc, tc4 * 512:(tc4 + 1) * 512],
                                    in_=k_ps)

                        # V1: token-major [keys, o] with ones column (col 64)
                        nc.vector.memset(V_sb, 1.0)
                        wv1_sb = wmain.tile([P, DT, D], BF16, tag="w2m", bufs=2)
                        nc.sync.dma_start(wv1_sb, wview(wv1T))
                        bv1 = bias_sb.get("bv1")
                        for kt in range(NKT):
                            for hc in range(2):
                                v_ps = pp.tile([P, 512], F32, tag="pp512", bufs=3)
                                for it in range(DT):
                                    nc.tensor.matmul(
                                        v_ps, ln1T[:, it, kt * P:(kt + 1) * P],
                                        wv1_sb[:, it, hc * 512:(hc + 1) * 512],
                                        start=(it == 0),
                                        stop=(it == DT - 1 and bv1 is None))
                                if bv1 is not None:
                                    nc.tensor.matmul(
                                        v_ps, ones_row.bitcast(BF16),
                                        bv1[:, hc * 512:(hc + 1) * 512],
                                        start=False, stop=True)
                                nc.scalar.copy(
                                    out=V_sb[:, kt, hc * 8:(hc + 1) * 8, 0:64],
                                    in_=v_ps.rearrange("p (h d) -> p h d", d=64))

                        # K2/V2 from context (independent of x)
                        ctx_sb = wpool.tile([P, CT, CN], BF16, tag="ctx", bufs=1)
                        for ct in range(CT):
                            nc.sync.dma_start(ctx_sb[:, ct, :], ctxT_v[ct])
                        wk2_sb = wpool.tile([P, CT, D], BF16, tag="w15", bufs=2)
                        nc.sync.dma_start(wk2_sb, wview(wk2T))
                        for oc in range(DT):
                            k_ps = pp.tile([P, CN], F32, tag="ppsm", bufs=2)
                            for it in range(CT):
                                nc.tensor.matmul(
                                    k_ps, wk2_sb[:, it, oc * P:(oc + 1) * P],
                                    ctx_sb[:, it, :],
                                    start=(it == 0), stop=(it == CT - 1))
                            nc.scalar.copy(out=K2_sb[:, oc, :], in_=k_ps)
                        nc.vector.memset(V2_sb, 1.0)
                        wv2_sb = wpool.tile([P, CT, D], BF16, tag="w15", bufs=2)
                        nc.sync.dma_start(wv2_sb, wview(wv2T))
                        for hc in range(2):
                            v_ps = pp.tile([CN, 512], F32, tag="ppsm", bufs=2)
                            for it in range(CT):
                                nc.tensor.matmul(
                                    v_ps, ctx_sb[:, it, :],
                                    wv2_sb[:, it, hc * 512:(hc + 1) * 512],
                                    start=(it == 0), stop=(it == CT - 1))
                            nc.scalar.copy(
                                out=V2_sb[0:CN, hc * 8:(hc + 1) * 8, 0:64],
                                in_=v_ps.rearrange("p (h d) -> p h d", d=64))
                    nc.leave_named_scope("phB_qkv", scopeB[0], False)

                # ----- Phase C: self-attention heads -----
                scopeC = nc.enter_named_scope("phC_attn", False)
                with tc.tile_pool(name="aps", bufs=1, space="PSUM") as apsum, \
                     tc.tile_pool(name="asb", bufs=1) as asb:
                    for h in range(H):
                        j, r0 = h >> 1, (h & 1) * 64
                        o_ps = apsum.tile([65, TO], F32, tag="o_ps", bufs=3)
                        for kt2 in range(NKT // 2):
                            # two key-tiles share one PSUM bank; second matmul
                            # overwrites its (cleared) half via has_written
                            s_ps = apsum.tile([P, 2, TO], F32, tag="s_ps", bufs=3)
                            for half in range(2):
                                kt = kt2 * 2 + half
                                nc.tensor.matmul(
                                    s_ps[:, half, :],
                                    K_sb[r0:r0 + 64, j, kt * P:(kt + 1) * P],
                                    Q_sb[r0:r0 + 64, j, :],
                                    start=(half == 0), stop=True,
                                    skip_group_check=(half == 1))
                            e_t = asb.tile([P, 2, TO], BF16, tag="e_t", bufs=6)
                            nc.scalar.activation(e_t, s_ps, AF.Exp, scale=SCALE)
                            for half in range(2):
                                kt = kt2 * 2 + half
                                nc.tensor.matmul(
                                    o_ps, V_sb[:, kt, h, :], e_t[:, half, :],
                                    start=(kt == 0), stop=(kt == NKT - 1))
                        r_sb = asb.tile([1, TO], F32R, tag="r_sb", bufs=4)
                        with nc.allow_low_precision("f32r == f32 bits"):
                            nc.vector.reciprocal(r_sb, o_ps[64:65, :])
                        r_ps = apsum.tile([64, TO], F32, tag="r_ps", bufs=2)
                        nc.tensor.matmul(r_ps, ones_row[:, :64], r_sb,
                                         start=True, stop=True)
                        r_bc = asb.tile([64, TO], F32, tag="r_bc", bufs=3)
                        nc.scalar.copy(out=r_bc, in_=r_ps)
                        nc.vector.tensor_tensor(out=O_sb[r0:r0 + 64, j, :],
                                                in0=o_ps[0:64, :],
                                                in1=r_bc, op=OP.mult)
                nc.leave_named_scope("phC_attn", scopeC[0], False)

                # ----- Phase D: attn1 out-proj + residual -----
                scopeD = nc.enter_named_scope("phD_oproj", False)
                with tc.tile_pool(name="dps", bufs=3, space="PSUM") as pp:
                    wo1_sb = wmain.tile([P, DT, D], BF16, tag="w2m", bufs=2)
                    nc.sync.dma_start(wo1_sb, wview(wo1T))

                    def add_residual(oc, ps):
                        nc.vector.tensor_tensor(
                            out=x_ownT[:, oc, :],
                            in0=x_ownT[:, oc, :].bitcast(F32),
                            in1=ps, op=OP.add)

                    proj_feature_major(pp, wo1_sb, O_sb, add_residual, DT, TO,
                                       bias=bias_sb.get("bo1"))
                nc.leave_named_scope("phD_oproj", scopeD[0], False)

            # ========== attn2 scope: phase E ==========
            scopeE = nc.enter_named_scope("phE_xattn", False)
            with tc.tile_pool(name="ce", bufs=1) as ce:
                ln2T = ce.tile([P, DT, TO], BF16)
                Q2_sb = ce.tile([P, DT, TO], BF16)
                O2_sb = ce.tile([P, DT, TO], BF16)

                with tc.tile_pool(name="lnps2", bufs=2, space="PSUM") as lnp, \
                     tc.tile_pool(name="lnsb2", bufs=2) as lnsb:
                    _ln_feature_major(
                        nc, lnp, lnsb, consts,
                        lambda dt, tci: x_ownT[:, dt, :],
                        lambda dt, tci: ln2T[:, dt, :],
                        DT, TO, TO)

                with tc.tile_pool(name="eps_", bufs=2, space="PSUM") as pp:
                    wq2_sb = wmain.tile([P, DT, D], BF16, tag="w2m", bufs=2)
                    nc.sync.dma_start(wq2_sb, wview(wq2T))
                    proj_feature_major(
                        pp, wq2_sb, ln2T,
                        lambda oc, ps: nc.scalar.copy(out=Q2_sb[:, oc, :],
                                                      in_=ps),
                        DT, TO, bias=bias_sb.get("bq2"))

                with tc.tile_pool(name="aps2", bufs=1, space="PSUM") as apsum, \
                     tc.tile_pool(name="asb2", bufs=1) as asb:
                    for h in range(H):
                        j, r0 = h >> 1, (h & 1) * 64
                        s_ps = apsum.tile([CN, TO], F32, tag="s_ps", bufs=3)
                        nc.tensor.matmul(
                            s_ps, K2_sb[r0:r0 + 64, j, :],
                            Q2_sb[r0:r0 + 64, j, :], start=True, stop=True)
                        e_t = asb.tile([CN, TO], BF16, tag="e_t", bufs=4)
                        nc.scalar.activation(e_t, s_ps, AF.Exp, scale=SCALE)
                        o_ps = apsum.tile([65, TO], F32, tag="o_ps", bufs=2)
                        nc.tensor.matmul(o_ps, V2_sb[0:CN, h, :], e_t,
                                         start=True, stop=True)
                        r_sb = asb.tile([1, TO], F32R, tag="r_sb", bufs=4)
                        with nc.allow_low_precision("f32r == f32 bits"):
                            nc.vector.reciprocal(r_sb, o_ps[64:65, :])
                        r_ps = apsum.tile([64, TO], F32, tag="r_ps", bufs=2)
                        nc.tensor.matmul(r_ps, ones_row[:, :64], r_sb,
                                         start=True, stop=True)
                        r_bc = asb.tile([64, TO], F32, tag="r_bc", bufs=3)
                        nc.scalar.copy(out=r_bc, in_=r_ps)
                        nc.vector.tensor_tensor(out=O2_sb[r0:r0 + 64, j, :],
                                                in0=o_ps[0:64, :],
                                                in1=r_bc, op=OP.mult)

                with tc.tile_pool(name="eps2", bufs=3, space="PSUM") as pp:
                    wo2_sb = wmain.tile([P, DT, D], BF16, tag="w2m", bufs=2)
                    nc.sync.dma_start(wo2_sb, wview(wo2T))

                    def add_residual2(oc, ps):
                        nc.vector.tensor_tensor(
                            out=x_ownT[:, oc, :],
                            in0=x_ownT[:, oc, :].bitcast(F32),
                            in1=ps, op=OP.add)

                    proj_feature_major(pp, wo2_sb, O2_sb, add_residual2, DT, TO,
                                       bias=bias_sb.get("bo2"))
            nc.leave_named_scope("phE_xattn", scopeE[0], False)

            # ========== FFN scope: phase F ==========
            scopeF = nc.enter_named_scope("phF_ffn", False)
            with tc.tile_pool(name="cf", bufs=1) as cf:
                ln3T = cf.tile([P, DT, TO], BF16)
                Hbuf = cf.tile([P, FT, TO], BF16)

                with tc.tile_pool(name="lnps3", bufs=2, space="PSUM") as lnp, \
                     tc.tile_pool(name="lnsb3", bufs=2) as lnsb:
                    _ln_feature_major(
                        nc, lnp, lnsb, consts,
                        lambda dt, tci: x_ownT[:, dt, :],
                        lambda dt, tci: ln3T[:, dt, :],
                        DT, TO, TO)

                wgT_v = wview(wgT)
                bgeg = bias_sb.get("bgeg")
                with tc.tile_pool(name="wg", bufs=1) as wgpool, \
                     tc.tile_pool(name="gps", bufs=1, space="PSUM") as gpsum, \
                     tc.tile_pool(name="gsb", bufs=3) as gsb:
                    for g in range(8):
                        wg_h = wgpool.tile([P, DT, 512], BF16, tag="wgh", bufs=2)
                        nc.sync.dma_start(wg_h,
                                          wgT_v[:, :, g * 512:(g + 1) * 512])
                        wg_g = wgpool.tile([P, DT, 512], BF16, tag="wgg", bufs=2)
                        nc.sync.dma_start(
                            wg_g, wgT_v[:, :, FF + g * 512:FF + (g + 1) * 512])
                        for fi in range(4):
                            f = g * 4 + fi
                            h_ps = gpsum.tile([P, TO], F32, tag="h_ps", bufs=2)
                            for it in range(DT):
                                nc.tensor.matmul(
                                    h_ps, wg_h[:, it, fi * P:(fi + 1) * P],
                                    ln3T[:, it, :],
                                    start=(it == 0),
                                    stop=(it == DT - 1 and bgeg is None))
                            if bgeg is not None:
                                nc.tensor.matmul(
                                    h_ps, bgeg[:, f * P:(f + 1) * P],
                                    ones_b[:, :TO], start=False, stop=True)
                            g_ps = gpsum.tile([P, TO], F32, tag="g_ps", bufs=2)
                            for it in range(DT):
                                nc.tensor.matmul(
                                    g_ps, wg_g[:, it, fi * P:(fi + 1) * P],
                                    ln3T[:, it, :],
                                    start=(it == 0),
                                    stop=(it == DT - 1 and bgeg is None))
                            if bgeg is not None:
                                nc.tensor.matmul(
                                    g_ps,
                                    bgeg[:, FF + f * P:FF + (f + 1) * P],
                                    ones_b[:, :TO], start=False, stop=True)
                            gel = gsb.tile([P, TO], F32, tag="gel", bufs=3)
                            nc.scalar.activation(gel, g_ps, AF.Gelu)
                            nc.vector.tensor_tensor(out=Hbuf[:, f, :],
                                                    in0=h_ps, in1=gel,
                                                    op=OP.mult)

                # ffout: two-level accumulation; spills add into x_ownT
                wfT_v = wfT.rearrange("(f p) o -> f p o", p=P)
                bff = bias_sb.get("bff")
                with tc.tile_pool(name="wfp", bufs=1) as wfpool, \
                     tc.tile_pool(name="yps", bufs=2, space="PSUM") as yp_:
                    for fg in range(4):
                        wf_tiles = []
                        for f8 in range(8):
                            wt = wfpool.tile([P, D], BF16, tag="wft", bufs=10)
                            nc.sync.dma_start(wt, wfT_v[fg * 8 + f8])
                            wf_tiles.append(wt)
                        for oc in range(DT):
                            i_ps = yp_.tile([P, TO], F32, tag="i_ps")
                            add_bias = bff is not None and fg == 3
                            for f8 in range(8):
                                nc.tensor.matmul(
                                    i_ps, wf_tiles[f8][:, oc * P:(oc + 1) * P],
                                    Hbuf[:, fg * 8 + f8, :],
                                    start=(f8 == 0),
                                    stop=(f8 == 7 and not add_bias))
                            if add_bias:
                                nc.tensor.matmul(
                                    i_ps, bff[:, oc * P:(oc + 1) * P],
                                    ones_b[:, :TO], start=False, stop=True)
                            nc.vector.tensor_tensor(
                                out=x_ownT[:, oc, :],
                                in0=x_ownT[:, oc, :].bitcast(F32),
                                in1=i_ps, op=OP.add)
                            if fg == 3:
                                nc.sync.dma_start(yT_v[:, oc, :],
                                                  x_ownT[:, oc, :])
            nc.leave_named_scope("phF_ffn", scopeF[0], False)

    nc.finalize()
    return nc


_CACHE = {}


def kernel(**inputs):
    def f32c(a):
        return np.ascontiguousarray(np.asarray(a, dtype=np.float32))

    def bfT(w):
        """W [out,in] (optionally gain-folded) -> bf16 W.T contiguous."""
        return np.ascontiguousarray(w.T).astype(ml_dtypes.bfloat16)

    x = f32c(inputs["hidden_states"])[0]          # [N, D]
    ctx = f32c(inputs["context"])[0]              # [CN, CD]
    g1 = f32c(inputs["ln1_g"]); b1 = f32c(inputs["ln1_b"])
    g2 = f32c(inputs["ln2_g"]); b2 = f32c(inputs["ln2_b"])
    g3 = f32c(inputs["ln3_g"]); b3 = f32c(inputs["ln3_b"])
    wq1 = f32c(inputs["wq1"]); wk1 = f32c(inputs["wk1"]); wv1 = f32c(inputs["wv1"])
    wo1 = f32c(inputs["wo1"]); bo1 = f32c(inputs["bo1"])
    wq2 = f32c(inputs["wq2"]); wk2 = f32c(inputs["wk2"]); wv2 = f32c(inputs["wv2"])
    wo2 = f32c(inputs["wo2"]); bo2 = f32c(inputs["bo2"])
    wg = f32c(inputs["w_geglu"]); bg = f32c(inputs["b_geglu"])
    wf = f32c(inputs["w_ffout"]); bf = f32c(inputs["b_ffout"])

    bq1 = wq1 @ b1; bk1 = wk1 @ b1; bv1 = wv1 @ b1
    bq2 = wq2 @ b2
    bgeg = bg + wg @ b3
    flags = (bool(np.any(bq1) or np.any(bk1) or np.any(bv1)), bool(np.any(bo1)),
             bool(np.any(bq2)), bool(np.any(bo2)), bool(np.any(bgeg)),
             bool(np.any(bf)))

    if flags not in _CACHE:
        _CACHE[flags] = build(flags)
    nc = _CACHE[flags]

    xT = np.ascontiguousarray(x.T)                # [D, N]
    bf16 = ml_dtypes.bfloat16
    shared = {
        "xT": xT,
        "ctxT": np.ascontiguousarray(ctx.T).astype(bf16),
        "wq1T": bfT(wq1 * g1[None, :]),
        "wk1T": bfT(wk1 * g1[None, :]),
        "wv1T": bfT(wv1 * g1[None, :]),
        "wo1T": bfT(wo1),
        "wq2T": bfT(wq2 * g2[None, :]),
        "wk2T": bfT(wk2),
        "wv2T": bfT(wv2),
        "wo2T": bfT(wo2),
        "wgT": bfT(wg * g3[None, :]),
        "wfT": bfT(wf),
        "onesc": np.ones((P, 1), np.float32),
        "onesr": np.ones((1, P), np.float32),
        "onesb": np.ones((1, 512), bf16),
    }
    if flags[0]:
        shared["bq1"] = bq1[None, :].astype(bf16)
        shared["bk1"] = bk1[None, :].astype(bf16)
        shared["bv1"] = bv1[None, :].astype(bf16)
    if flags[1]:
        shared["bo1"] = bo1[None, :].astype(bf16)
    if flags[2]:
        shared["bq2"] = bq2[None, :].astype(bf16)
    if flags[3]:
        shared["bo2"] = bo2[None, :].astype(bf16)
    if flags[4]:
        shared["bgeg"] = bgeg[None, :].astype(bf16)
    if flags[5]:
        shared["bff"] = bf[None, :].astype(bf16)

    in_maps = []
    for c in range(NCORES):
        m = dict(shared)
        m["xoT"] = np.ascontiguousarray(xT[:, c * TO:(c + 1) * TO])
        in_maps.append(m)

    res = run_bass_kernel_spmd(nc, in_maps, core_ids=list(range(NCORES)))
    yT = np.concatenate([r["yT"] for r in res.results], axis=1)  # [D, N]
    return np.ascontiguousarray(yT.T)[None].astype(np.float32)


# revision 19
# speedup vs baseline: 1.0971x; 1.0115x over previous
"""Trainium2 Bass kernel for nn_BasicTransformerBlock (self-attn + cross-attn + GEGLU).

Sharding: data-parallel over the 2048 tokens (256 per core, 8 cores, no
collectives). K/V for self-attention are computed replicated on every core.

On-chip layout is feature-major throughout ([feature(part), token(free)]).
Host pre-packs weights as bf16 W.T (C-contiguous [in, out]) and pre-transposes
x / context, so the device does zero transposes/casts and all DMAs are
contiguous. Weight/projection matmuls run in bf16 (fp32 PSUM accumulate);
LayerNorm statistics run in float32r off the fp32 residual stream.

Softmax: scores are computed keys-on-partitions ([keys, q]); exp on ScalarE
with the 1/8 scale folded in; denominators come from an appended ones-column
in V (row 64 of the AV accumulation); all 16 head sums are staged into one
tile, inverted with a single reciprocal, and divided in via a PE-broadcast.
"""

import numpy as np
import ml_dtypes

import concourse.bass as bass
import concourse.mybir as mybir
import concourse.tile as tile
from concourse import bacc
from concourse.bass_utils import run_bass_kernel_spmd

F32 = mybir.dt.float32
F32R = mybir.dt.float32r
BF16 = mybir.dt.bfloat16
AF = mybir.ActivationFunctionType
OP = mybir.AluOpType

P = 128
N, D = 2048, 1024
H, DH = 16, 64
CN, CD = 77, 768
FF = 4096
EPS = 1e-5
SCALE = DH ** -0.5
NCORES = 8
TO = N // NCORES          # 256 tokens owned per core
DT = D // P               # 8 feature tiles
CT = CD // P              # 6 context-feature tiles
NKT = N // P              # 16 key tiles
FT = FF // P              # 32 ffn-inner tiles


def _ln_feature_major(nc, lnp, sbp, consts, src_of, dst_of, n_dt, tn, chunk,
                      post_cb=None):
    """Un-affine LayerNorm over feature-major f32r data."""
    ones_col, ones_row, eps_t = consts
    inv_d = 1.0 / (n_dt * P)
    for tci in range(tn // chunk):
        srcs = [src_of(dt, tci) for dt in range(n_dt)]   # f32r tiles
        sum_ps = lnp.tile([1, chunk], F32, tag="ln_sum", bufs=2)
        for dt in range(n_dt):
            nc.tensor.matmul(sum_ps, ones_col, srcs[dt],
                             start=(dt == 0), stop=(dt == n_dt - 1))
        sumsq_ps = lnp.tile([1, chunk], F32, tag="ln_sumsq", bufs=2)
        for dt in range(n_dt):
            sq_t = sbp.tile([P, chunk], F32R, tag="ln_sq", bufs=2)
            nc.scalar.activation(sq_t, srcs[dt].bitcast(F32), AF.Square)
            nc.tensor.matmul(sumsq_ps, ones_col, sq_t,
                             start=(dt == 0), stop=(dt == n_dt - 1))
        mu_row = sbp.tile([1, chunk], F32R, tag="ln_mu", bufs=2)
        nc.scalar.mul(out=mu_row, in_=sum_ps, mul=inv_d)
        var_row = sbp.tile([1, chunk], F32, tag="ln_var", bufs=2)
        nc.scalar.mul(out=var_row, in_=sumsq_ps, mul=inv_d)
        musq = sbp.tile([1, chunk], F32, tag="ln_musq", bufs=2)
        nc.vector.tensor_mul(out=musq, in0=mu_row.bitcast(F32),
                             in1=mu_row.bitcast(F32))
        nc.vector.tensor_tensor(out=var_row, in0=var_row, in1=musq,
                                op=OP.subtract)
        nc.scalar.activation(var_row, var_row, AF.Sqrt, bias=eps_t)
        rstd_row = sbp.tile([1, chunk], F32R, tag="ln_rstd", bufs=2)
        with nc.allow_low_precision("f32r keeps full fp32 bits here"):
            nc.vector.reciprocal(rstd_row, var_row)
        mu_b = lnp.tile([P, chunk], F32, tag="ln_mub", bufs=1)
        nc.tensor.matmul(mu_b, ones_row, mu_row, start=True, stop=True)
        rstd_b = lnp.tile([P, chunk], F32, tag="ln_rstdb", bufs=1)
        nc.tensor.matmul(rstd_b, ones_row, rstd_row, start=True, stop=True)
        mu_s = sbp.tile([P, chunk], F32, tag="ln_mus", bufs=2)
        nc.scalar.copy(out=mu_s, in_=mu_b)
        for dt in range(n_dt):
            tmp = sbp.tile([P, chunk], F32, tag="ln_tmp", bufs=3)
            nc.gpsimd.tensor_tensor(out=tmp, in0=srcs[dt].bitcast(F32),
                                    in1=mu_s, op=OP.subtract)
            nc.vector.tensor_tensor(out=dst_of(dt, tci), in0=tmp, in1=rstd_b,
                                    op=OP.mult)
        if post_cb is not None:
            post_cb(tci)


def build(flags):
    has_qkv1b, has_bo1, has_q2b, has_bo2, has_gegb, has_ffb = flags
    nc = bacc.Bacc()

    xT = nc.dram_tensor("xT", [D, N], F32R, kind="ExternalInput")
    xoT = nc.dram_tensor("xoT", [D, TO], F32R, kind="ExternalInput")
    ctxT = nc.dram_tensor("ctxT", [CD, CN], BF16, kind="ExternalInput")
    wq1T = nc.dram_tensor("wq1T", [D, D], BF16, kind="ExternalInput")
    wk1T = nc.dram_tensor("wk1T", [D, D], BF16, kind="ExternalInput")
    wv1T = nc.dram_tensor("wv1T", [D, D], BF16, kind="ExternalInput")
    wo1T = nc.dram_tensor("wo1T", [D, D], BF16, kind="ExternalInput")
    wq2T = nc.dram_tensor("wq2T", [D, D], BF16, kind="ExternalInput")
    wk2T = nc.dram_tensor("wk2T", [CD, D], BF16, kind="ExternalInput")
    wv2T = nc.dram_tensor("wv2T", [CD, D], BF16, kind="ExternalInput")
    wo2T = nc.dram_tensor("wo2T", [D, D], BF16, kind="ExternalInput")
    wgT = nc.dram_tensor("wgT", [D, 2 * FF], BF16, kind="ExternalInput")
    wfT = nc.dram_tensor("wfT", [FF, D], BF16, kind="ExternalInput")
    onesc = nc.dram_tensor("onesc", [P, 1], F32R, kind="ExternalInput")
    onesr = nc.dram_tensor("onesr", [1, P], F32R, kind="ExternalInput")
    onesb = nc.dram_tensor("onesb", [1, 512], BF16, kind="ExternalInput")
    bias_rows = {}
    if has_qkv1b:
        for nm in ("bq1", "bk1", "bv1"):
            bias_rows[nm] = nc.dram_tensor(nm, [1, D], BF16, kind="ExternalInput")
    if has_bo1:
        bias_rows["bo1"] = nc.dram_tensor("bo1", [1, D], BF16, kind="ExternalInput")
    if has_q2b:
        bias_rows["bq2"] = nc.dram_tensor("bq2", [1, D], BF16, kind="ExternalInput")
    if has_bo2:
        bias_rows["bo2"] = nc.dram_tensor("bo2", [1, D], BF16, kind="ExternalInput")
    if has_gegb:
        bias_rows["bgeg"] = nc.dram_tensor("bgeg", [1, 2 * FF], BF16,
                                           kind="ExternalInput")
    if has_ffb:
        bias_rows["bff"] = nc.dram_tensor("bff", [1, D], BF16, kind="ExternalInput")
    yT = nc.dram_tensor("yT", [D, TO], F32R, kind="ExternalOutput")

    xT_v = xT.rearrange("(dt p) t -> dt p t", p=P)
    xoT_v = xoT.rearrange("(dt p) t -> dt p t", p=P)
    ctxT_v = ctxT.rearrange("(ct p) t -> ct p t", p=P)
    yT_v = yT.rearrange("(dt p) t -> p dt t", p=P)

    def wview(w):
        return w.rearrange("(it p) o -> p it o", p=P)

    with tile.TileContext(nc) as tc:
        with tc.tile_pool(name="consts", bufs=1) as cpool, \
             tc.tile_pool(name="pers", bufs=1) as pers, \
             tc.tile_pool(name="wmain", bufs=1) as wmain:

            ones_col = cpool.tile([P, 1], F32R)
            nc.sync.dma_start(ones_col, onesc[:])
            ones_row = cpool.tile([1, P], F32R)
            nc.sync.dma_start(ones_row, onesr[:])
            ones_b = cpool.tile([1, 512], BF16)
            nc.sync.dma_start(ones_b, onesb[:])
            eps_t = cpool.tile([1, 1], F32)
            nc.vector.memset(eps_t, EPS)
            consts = (ones_col, ones_row, eps_t)

            bias_sb = {}
            for nm, t in bias_rows.items():
                bt = cpool.tile([1, t.shape[1]], BF16, tag=f"bias_{nm}")
                nc.sync.dma_start(bt, t[:])
                bias_sb[nm] = bt

            def proj_feature_major(pp, w_sb, act, out_cb, n_in, n_tok,
                                   bias=None, tag="pp256"):
                """out[oc] = sum_it w.T @ act; out_cb(oc, psum)."""
                for oc in range(DT):
                    ps = pp.tile([P, n_tok], F32, tag=tag, bufs=2)
                    for it in range(n_in):
                        nc.tensor.matmul(ps, w_sb[:, it, oc * P:(oc + 1) * P],
                                         act[:, it, :],
                                         start=(it == 0),
                                         stop=(it == n_in - 1 and bias is None))
                    if bias is not None:
                        nc.tensor.matmul(ps, bias[:, oc * P:(oc + 1) * P],
                                         ones_b[:, :n_tok], start=False,
                                         stop=True)
                    out_cb(oc, ps)

            x_ownT = pers.tile([P, DT, TO], F32R)      # residual stream (own)
            for dt in range(DT):
                nc.sync.dma_start(x_ownT[:, dt, :], xoT_v[dt])

            # cross-attn K2/V2 depend only on the context: computed early in
            # phase B so they overlap everything up to phase E.
            K2_sb = pers.tile([P, DT, CN], BF16)
            V2_sb = pers.tile([P, H, 65], BF16)

            # ========== attn1 scope: phases A-D ==========
            with tc.tile_pool(name="c1", bufs=1) as c1:
                O_sb = c1.tile([P, DT, TO], BF16)
                K_sb = c1.tile([P, DT, N], BF16)
                V_sb = c1.tile([P, NKT, H, 65], BF16)
                Q_sb = c1.tile([P, DT, TO], BF16)
                lnoT = c1.tile([P, DT, TO], BF16)

                with tc.tile_pool(name="c2", bufs=1) as c2:
                    ln1T = c2.tile([P, DT, N], BF16)

                    # ----- Phase A+B fused: LN1 chunk -> K/V for that chunk --
                    scopeA = nc.enter_named_scope("phA_ln1", False)
                    LCH = 512
                    # weight prefetches first: DMA runs under LN compute
                    wq1_sb = wmain.tile([P, DT, D], BF16, tag="w2m", bufs=2)
                    nc.sync.dma_start(wq1_sb, wview(wq1T))
                    wk1_sb = wmain.tile([P, DT, D], BF16, tag="w2m", bufs=2)
                    nc.sync.dma_start(wk1_sb, wview(wk1T))
                    bk1 = bias_sb.get("bk1")
                    bv1 = bias_sb.get("bv1")
                    with tc.tile_pool(name="lnps", bufs=2, space="PSUM") as lnp, \
                         tc.tile_pool(name="lnsb", bufs=2) as lnsb:
                        with tc.tile_pool(name="qps", bufs=2,
                                          space="PSUM") as qpp:
                            _ln_feature_major(
                                nc, lnp, lnsb, consts,
                                lambda dt, tci: x_ownT[:, dt, :],
                                lambda dt, tci: lnoT[:, dt, :],
                                DT, TO, TO)
                            proj_feature_major(
                                qpp, wq1_sb, lnoT,
                                lambda oc, ps: nc.scalar.copy(
                                    out=Q_sb[:, oc, :], in_=ps),
                                DT, TO, bias=bias_sb.get("bq1"))
                        nc.vector.memset(V_sb, 1.0)
                        with tc.tile_pool(name="projps", bufs=2,
                                          space="PSUM") as pp:
                            wv1_sb = wmain.tile([P, DT, D], BF16, tag="w2m",
                                                bufs=2)
                            nc.sync.dma_start(wv1_sb, wview(wv1T))

                            def kv_for_chunk(tci):
                                for oc in range(DT):
                                    k_ps = pp.tile([P, 512], F32, tag="pp512",
                                                   bufs=2)
                                    for it in range(DT):
                                        nc.tensor.matmul(
                                            k_ps,
                                            wk1_sb[:, it, oc * P:(oc + 1) * P],
                                            ln1T[:, it,
                                                 tci * 512:(tci + 1) * 512],
                                            start=(it == 0),
                                            stop=(it == DT - 1 and bk1 is None))
                                    if bk1 is not None:
                                        nc.tensor.matmul(
                                            k_ps, bk1[:, oc * P:(oc + 1) * P],
                                            ones_b, start=False, stop=True)
                                    nc.vector.tensor_copy(
                                        out=K_sb[:, oc,
                                                 tci * 512:(tci + 1) * 512],
                                        in_=k_ps)
                                for kt in range(tci * 4, tci * 4 + 4):
                                    for hc in range(2):
                                        v_ps = pp.tile([P, 512], F32,
                                                       tag="pp512", bufs=2)
                                        for it in range(DT):
                                            nc.tensor.matmul(
                                                v_ps,
                                                ln1T[:, it, kt * P:(kt + 1) * P],
                                                wv1_sb[:, it,
                                                       hc * 512:(hc + 1) * 512],
                                                start=(it == 0),
                                                stop=(it == DT - 1
                                                      and bv1 is None))
                                        if bv1 is not None:
                                            nc.tensor.matmul(
                                                v_ps, ones_row.bitcast(BF16),
                                                bv1[:, hc * 512:(hc + 1) * 512],
                                                start=False, stop=True)
                                        nc.scalar.copy(
                                            out=V_sb[:, kt,
                                                     hc * 8:(hc + 1) * 8, 0:64],
                                            in_=v_ps.rearrange(
                                                "p (h d) -> p h d", d=64))

                            def load_x(dt, tci, _c={}):
                                if (dt, tci) not in _c:
                                    t = lnsb.tile([P, LCH], F32R, tag="xt",
                                                  bufs=9)
                                    nc.sync.dma_start(
                                        t,
                                        xT_v[dt, :, tci * LCH:(tci + 1) * LCH])
                                    _c[(dt, tci)] = t
                                return _c[(dt, tci)]

                            _ln_feature_major(
                                nc, lnp, lnsb, consts, load_x,
                                lambda dt, tci: ln1T[:, dt,
                                                     tci * LCH:(tci + 1) * LCH],
                                DT, N, LCH, post_cb=kv_for_chunk)
                    nc.leave_named_scope("phA_ln1", scopeA[0], False)

                    # ----- Phase B remainder: K2/V2 (context) -----
                    scopeB = nc.enter_named_scope("phB_qkv", False)
                    with tc.tile_pool(name="wb", bufs=1) as wpool, \
                         tc.tile_pool(name="projps2", bufs=2, space="PSUM") as pp:
                        # K2/V2 from context (independent of x)
                        ctx_sb = wpool.tile([P, CT, CN], BF16, tag="ctx", bufs=1)
                        for ct in range(CT):
                            nc.sync.dma_start(ctx_sb[:, ct, :], ctxT_v[ct])
                        wk2_sb = wpool.tile([P, CT, D], BF16, tag="w15", bufs=2)
                        nc.sync.dma_start(wk2_sb, wview(wk2T))
                        for oc in range(DT):
                            k_ps = pp.tile([P, CN], F32, tag="ppsm", bufs=2)
                            for it in range(CT):
                                nc.tensor.matmul(
                                    k_ps, wk2_sb[:, it, oc * P:(oc + 1) * P],
                                    ctx_sb[:, it, :],
                                    start=(it == 0), stop=(it == CT - 1))
                            nc.scalar.copy(out=K2_sb[:, oc, :], in_=k_ps)
                        nc.vector.memset(V2_sb, 1.0)
                        wv2_sb = wpool.tile([P, CT, D], BF16, tag="w15", bufs=2)
                        nc.sync.dma_start(wv2_sb, wview(wv2T))
                        for hc in range(2):
                            v_ps = pp.tile([CN, 512], F32, tag="ppsm", bufs=2)
                            for it in range(CT):
                                nc.tensor.matmul(
                                    v_ps, ctx_sb[:, it, :],
                                    wv2_sb[:, it, hc * 512:(hc + 1) * 512],
                                    start=(it == 0), stop=(it == CT - 1))
                            nc.scalar.copy(
                                out=V2_sb[0:CN, hc * 8:(hc + 1) * 8, 0:64],
                                in_=v_ps.rearrange("p (h d) -> p h d", d=64))
                    nc.leave_named_scope("phB_qkv", scopeB[0], False)

                # ----- Phase C: self-attention heads -----
                scopeC = nc.enter_named_scope("phC_attn", False)
                with tc.tile_pool(name="aps", bufs=1, space="PSUM") as apsum, \
                     tc.tile_pool(name="asb", bufs=1) as asb:
                    for h in range(H):
                        j, r0 = h >> 1, (h & 1) * 64
                        o_ps = apsum.tile([65, TO], F32, tag="o_ps", bufs=3)
                        for kt2 in range(NKT // 2):
                            # two key-tiles share one PSUM bank; second matmul
                            # overwrites its (cleared) half via has_written
                            s_ps = apsum.tile([P, 2, TO], F32, tag="s_ps", bufs=3)
                            for half in range(2):
                                kt = kt2 * 2 + half
                                nc.tensor.matmul(
                                    s_ps[:, half, :],
                                    K_sb[r0:r0 + 64, j, kt * P:(kt + 1) * P],
                                    Q_sb[r0:r0 + 64, j, :],
                                    start=(half == 0), stop=True,
                                    skip_group_check=(half == 1))
                            e_t = asb.tile([P, 2, TO], BF16, tag="e_t", bufs=6)
                            nc.scalar.activation(e_t, s_ps, AF.Exp, scale=SCALE)
                            for half in range(2):
                                kt = kt2 * 2 + half
                                nc.tensor.matmul(
                                    o_ps, V_sb[:, kt, h, :], e_t[:, half, :],
                                    start=(kt == 0), stop=(kt == NKT - 1))
                        r_sb = asb.tile([1, TO], F32R, tag="r_sb", bufs=4)
                        with nc.allow_low_precision("f32r == f32 bits"):
                            nc.vector.reciprocal(r_sb, o_ps[64:65, :])
                        r_ps = apsum.tile([64, TO], F32, tag="r_ps", bufs=2)
                        nc.tensor.matmul(r_ps, ones_row[:, :64], r_sb,
                                         start=True, stop=True)
                        r_bc = asb.tile([64, TO], F32, tag="r_bc", bufs=3)
                        nc.scalar.copy(out=r_bc, in_=r_ps)
                        nc.vector.tensor_tensor(out=O_sb[r0:r0 + 64, j, :],
                                                in0=o_ps[0:64, :],
                                                in1=r_bc, op=OP.mult)
                nc.leave_named_scope("phC_attn", scopeC[0], False)

                # ----- Phase D: attn1 out-proj + residual -----
                scopeD = nc.enter_named_scope("phD_oproj", False)
                with tc.tile_pool(name="dps", bufs=3, space="PSUM") as pp:
                    wo1_sb = wmain.tile([P, DT, D], BF16, tag="w2m", bufs=2)
                    nc.sync.dma_start(wo1_sb, wview(wo1T))

                    def add_residual(oc, ps):
                        nc.vector.tensor_tensor(
                            out=x_ownT[:, oc, :],
                            in0=x_ownT[:, oc, :].bitcast(F32),
                            in1=ps, op=OP.add)

                    proj_feature_major(pp, wo1_sb, O_sb, add_residual, DT, TO,
                                       bias=bias_sb.get("bo1"))
                nc.leave_named_scope("phD_oproj", scopeD[0], False)

            # ========== attn2 scope: phase E ==========
            scopeE = nc.enter_named_scope("phE_xattn", False)
            with tc.tile_pool(name="ce", bufs=1) as ce:
                ln2T = ce.tile([P, DT, TO], BF16)
                Q2_sb = ce.tile([P, DT, TO], BF16)
                O2_sb = ce.tile([P, DT, TO], BF16)

                with tc.tile_pool(name="lnps2", bufs=2, space="PSUM") as lnp, \
                     tc.tile_pool(name="lnsb2", bufs=2) as lnsb:
                    _ln_feature_major(
                        nc, lnp, lnsb, consts,
                        lambda dt, tci: x_ownT[:, dt, :],
                        lambda dt, tci: ln2T[:, dt, :],
                        DT, TO, TO)

                with tc.tile_pool(name="eps_", bufs=2, space="PSUM") as pp:
                    wq2_sb = wmain.tile([P, DT, D], BF16, tag="w2m", bufs=2)
                    nc.sync.dma_start(wq2_sb, wview(wq2T))
                    proj_feature_major(
                        pp, wq2_sb, ln2T,
                        lambda oc, ps: nc.scalar.copy(out=Q2_sb[:, oc, :],
                                                      in_=ps),
                        DT, TO, bias=bias_sb.get("bq2"))

                with tc.tile_pool(name="aps2", bufs=1, space="PSUM") as apsum, \
                     tc.tile_pool(name="asb2", bufs=1) as asb:
                    for h in range(H):
                        j, r0 = h >> 1, (h & 1) * 64
                        s_ps = apsum.tile([CN, TO], F32, tag="s_ps", bufs=3)
                        nc.tensor.matmul(
                            s_ps, K2_sb[r0:r0 + 64, j, :],
                            Q2_sb[r0:r0 + 64, j, :], start=True, stop=True)
                        e_t = asb.tile([CN, TO], BF16, tag="e_t", bufs=4)
                        nc.scalar.activation(e_t, s_ps, AF.Exp, scale=SCALE)
                        o_ps = apsum.tile([65, TO], F32, tag="o_ps", bufs=2)
                        nc.tensor.matmul(o_ps, V2_sb[0:CN, h, :], e_t,
                                         start=True, stop=True)
                        r_sb = asb.tile([1, TO], F32R, tag="r_sb", bufs=4)
                        with nc.allow_low_precision("f32r == f32 bits"):
                            nc.vector.reciprocal(r_sb, o_ps[64:65, :])
                        r_ps = apsum.tile([64, TO], F32, tag="r_ps", bufs=2)
                        nc.tensor.matmul(r_ps, ones_row[:, :64], r_sb,
                                         start=True, stop=True)
                        r_bc = asb.tile([64, TO], F32, tag="r_bc", bufs=3)
                        nc.scalar.copy(out=r_bc, in_=r_ps)
                        nc.vector.tensor_tensor(out=O2_sb[r0:r0 + 64, j, :],
                                                in0=o_ps[0:64, :],
                                                in1=r_bc, op=OP.mult)

                with tc.tile_pool(name="eps2", bufs=3, space="PSUM") as pp:
                    wo2_sb = wmain.tile([P, DT, D], BF16, tag="w2m", bufs=2)
                    nc.sync.dma_start(wo2_sb, wview(wo2T))

                    def add_residual2(oc, ps):
                        nc.vector.tensor_tensor(
                            out=x_ownT[:, oc, :],
                            in0=x_ownT[:, oc, :].bitcast(F32),
                            in1=ps, op=OP.add)

                    proj_feature_major(pp, wo2_sb, O2_sb, add_residual2, DT, TO,
                                       bias=bias_sb.get("bo2"))
            nc.leave_named_scope("phE_xattn", scopeE[0], False)

            # ========== FFN scope: phase F ==========
            scopeF = nc.enter_named_scope("phF_ffn", False)
            with tc.tile_pool(name="cf", bufs=1) as cf:
                ln3T = cf.tile([P, DT, TO], BF16)
                Hbuf = cf.tile([P, FT, TO], BF16)

                with tc.tile_pool(name="lnps3", bufs=2, space="PSUM") as lnp, \
                     tc.tile_pool(name="lnsb3", bufs=2) as lnsb:
                    _ln_feature_major(
                        nc, lnp, lnsb, consts,
                        lambda dt, tci: x_ownT[:, dt, :],
                        lambda dt, tci: ln3T[:, dt, :],
                        DT, TO, TO)

                wgT_v = wview(wgT)
                bgeg = bias_sb.get("bgeg")
                with tc.tile_pool(name="wg", bufs=1) as wgpool, \
                     tc.tile_pool(name="gps", bufs=1, space="PSUM") as gpsum, \
                     tc.tile_pool(name="gsb", bufs=3) as gsb:
                    for g in range(8):
                        wg_h = wgpool.tile([P, DT, 512], BF16, tag="wgh", bufs=2)
                        nc.sync.dma_start(wg_h,
                                          wgT_v[:, :, g * 512:(g + 1) * 512])
                        wg_g = wgpool.tile([P, DT, 512], BF16, tag="wgg", bufs=2)
                        nc.sync.dma_start(
                            wg_g, wgT_v[:, :, FF + g * 512:FF + (g + 1) * 512])
                        for fi in range(4):
                            f = g * 4 + fi
                            h_ps = gpsum.tile([P, TO], F32, tag="h_ps", bufs=2)
                            for it in range(DT):
                                nc.tensor.matmul(
                                    h_ps, wg_h[:, it, fi * P:(fi + 1) * P],
                                    ln3T[:, it, :],
                                    start=(it == 0),
                                    stop=(it == DT - 1 and bgeg is None))
                            if bgeg is not None:
                                nc.tensor.matmul(
                                    h_ps, bgeg[:, f * P:(f + 1) * P],
                                    ones_b[:, :TO], start=False, stop=True)
                            g_ps = gpsum.tile([P, TO], F32, tag="g_ps", bufs=2)
                            for it in range(DT):
                                nc.tensor.matmul(
                                    g_ps, wg_g[:, it, fi * P:(fi + 1) * P],
                                    ln3T[:, it, :],
                                    start=(it == 0),
                                    stop=(it == DT - 1 and bgeg is None))
                            if bgeg is not None:
                                nc.tensor.matmul(
                                    g_ps,
                                    bgeg[:, FF + f * P:FF + (f + 1) * P],
                                    ones_b[:, :TO], start=False, stop=True)
                            gel = gsb.tile([P, TO], F32, tag="gel", bufs=3)
                            nc.scalar.activation(gel, g_ps, AF.Gelu)
                            nc.vector.tensor_tensor(out=Hbuf[:, f, :],
                                                    in0=h_ps, in1=gel,
                                                    op=OP.mult)

                # ffout: two-level accumulation; spills add into x_ownT
                wfT_v = wfT.rearrange("(f p) o -> f p o", p=P)
                bff = bias_sb.get("bff")
                with tc.tile_pool(name="wfp", bufs=1) as wfpool, \
                     tc.tile_pool(name="yps", bufs=2, space="PSUM") as yp_:
                    for fg in range(4):
                        wf_tiles = []
                        for f8 in range(8):
                            wt = wfpool.tile([P, D], BF16, tag="wft", bufs=10)
                            nc.sync.dma_start(wt, wfT_v[fg * 8 + f8])
                            wf_tiles.append(wt)
                        for oc in range(DT):
                            i_ps = yp_.tile([P, TO], F32, tag="i_ps")
                            add_bias = bff is not None and fg == 3
                            for f8 in range(8):
                                nc.tensor.matmul(
                                    i_ps, wf_tiles[f8][:, oc * P:(oc + 1) * P],
                                    Hbuf[:, fg * 8 + f8, :],
                                    start=(f8 == 0),
                                    stop=(f8 == 7 and not add_bias))
                            if add_bias:
                                nc.tensor.matmul(
                                    i_ps, bff[:, oc * P:(oc + 1) * P],
                                    ones_b[:, :TO], start=False, stop=True)
                            nc.vector.tensor_tensor(
                                out=x_ownT[:, oc, :],
                                in0=x_ownT[:, oc, :].bitcast(F32),
                                in1=i_ps, op=OP.add)
                            if fg == 3:
                                nc.sync.dma_start(yT_v[:, oc, :],
                                                  x_ownT[:, oc, :])
            nc.leave_named_scope("phF_ffn", scopeF[0], False)

    nc.finalize()
    return nc


_CACHE = {}


def kernel(**inputs):
    def f32c(a):
        return np.ascontiguousarray(np.asarray(a, dtype=np.float32))

    def bfT(w):
        """W [out,in] (optionally gain-folded) -> bf16 W.T contiguous."""
        return np.ascontiguousarray(w.T).astype(ml_dtypes.bfloat16)

    x = f32c(inputs["hidden_states"])[0]          # [N, D]
    ctx = f32c(inputs["context"])[0]              # [CN, CD]
    g1 = f32c(inputs["ln1_g"]); b1 = f32c(inputs["ln1_b"])
    g2 = f32c(inputs["ln2_g"]); b2 = f32c(inputs["ln2_b"])
    g3 = f32c(inputs["ln3_g"]); b3 = f32c(inputs["ln3_b"])
    wq1 = f32c(inputs["wq1"]); wk1 = f32c(inputs["wk1"]); wv1 = f32c(inputs["wv1"])
    wo1 = f32c(inputs["wo1"]); bo1 = f32c(inputs["bo1"])
    wq2 = f32c(inputs["wq2"]); wk2 = f32c(inputs["wk2"]); wv2 = f32c(inputs["wv2"])
    wo2 = f32c(inputs["wo2"]); bo2 = f32c(inputs["bo2"])
    wg = f32c(inputs["w_geglu"]); bg = f32c(inputs["b_geglu"])
    wf = f32c(inputs["w_ffout"]); bf = f32c(inputs["b_ffout"])

    bq1 = wq1 @ b1; bk1 = wk1 @ b1; bv1 = wv1 @ b1
    bq2 = wq2 @ b2
    bgeg = bg + wg @ b3
    flags = (bool(np.any(bq1) or np.any(bk1) or np.any(bv1)), bool(np.any(bo1)),
             bool(np.any(bq2)), bool(np.any(bo2)), bool(np.any(bgeg)),
             bool(np.any(bf)))

    if flags not in _CACHE:
        _CACHE[flags] = build(flags)
    nc = _CACHE[flags]

    xT = np.ascontiguousarray(x.T)                # [D, N]
    bf16 = ml_dtypes.bfloat16
    shared = {
        "xT": xT,
        "ctxT": np.ascontiguousarray(ctx.T).astype(bf16),
        "wq1T": bfT(wq1 * g1[None, :]),
        "wk1T": bfT(wk1 * g1[None, :]),
        "wv1T": bfT(wv1 * g1[None, :]),
        "wo1T": bfT(wo1),
        "wq2T": bfT(wq2 * g2[None, :]),
        "wk2T": bfT(wk2),
        "wv2T": bfT(wv2),
        "wo2T": bfT(wo2),
        "wgT": bfT(wg * g3[None, :]),
        "wfT": bfT(wf),
        "onesc": np.ones((P, 1), np.float32),
        "onesr": np.ones((1, P), np.float32),
        "onesb": np.ones((1, 512), bf16),
    }
    if flags[0]:
        shared["bq1"] = bq1[None, :].astype(bf16)
        shared["bk1"] = bk1[None, :].astype(bf16)
        shared["bv1"] = bv1[None, :].astype(bf16)
    if flags[1]:
        shared["bo1"] = bo1[None, :].astype(bf16)
    if flags[2]:
        shared["bq2"] = bq2[None, :].astype(bf16)
    if flags[3]:
        shared["bo2"] = bo2[None, :].astype(bf16)
    if flags[4]:
        shared["bgeg"] = bgeg[None, :].astype(bf16)
    if flags[5]:
        shared["bff"] = bf[None, :].astype(bf16)

    in_maps = []
    for c in range(NCORES):
        m = dict(shared)
        m["xoT"] = np.ascontiguousarray(xT[:, c * TO:(c + 1) * TO])
        in_maps.append(m)

    res = run_bass_kernel_spmd(nc, in_maps, core_ids=list(range(NCORES)))
    yT = np.concatenate([r["yT"] for r in res.results], axis=1)  # [D, N]
    return np.ascontiguousarray(yT.T)[None].astype(np.float32)


# revision 21
# speedup vs baseline: 1.1170x; 1.0181x over previous
"""Trainium2 Bass kernel for nn_BasicTransformerBlock (self-attn + cross-attn + GEGLU).

Sharding: data-parallel over the 2048 tokens (256 per core, 8 cores, no
collectives). K/V for self-attention are computed replicated on every core.

On-chip layout is feature-major throughout ([feature(part), token(free)]).
Host pre-packs weights as bf16 W.T (C-contiguous [in, out]) and pre-transposes
x / context, so the device does zero transposes/casts and all DMAs are
contiguous. Weight/projection matmuls run in bf16 (fp32 PSUM accumulate);
LayerNorm statistics run in float32r off the fp32 residual stream.

Softmax: scores are computed keys-on-partitions ([keys, q]); exp on ScalarE
with the 1/8 scale folded in; denominators come from an appended ones-column
in V (row 64 of the AV accumulation); all 16 head sums are staged into one
tile, inverted with a single reciprocal, and divided in via a PE-broadcast.
"""

import numpy as np
import ml_dtypes

import concourse.bass as bass
import concourse.mybir as mybir
import concourse.tile as tile
from concourse import bacc
from concourse.bass_utils import run_bass_kernel_spmd

F32 = mybir.dt.float32
F32R = mybir.dt.float32r
BF16 = mybir.dt.bfloat16
AF = mybir.ActivationFunctionType
OP = mybir.AluOpType

P = 128
N, D = 2048, 1024
H, DH = 16, 64
CN, CD = 77, 768
FF = 4096
EPS = 1e-5
SCALE = DH ** -0.5
NCORES = 8
TO = N // NCORES          # 256 tokens owned per core
DT = D // P               # 8 feature tiles
CT = CD // P              # 6 context-feature tiles
NKT = N // P              # 16 key tiles
FT = FF // P              # 32 ffn-inner tiles


def _ln_feature_major(nc, lnp, sbp, consts, src_of, dst_of, n_dt, tn, chunk,
                      post_cb=None):
    """Un-affine LayerNorm over feature-major f32r data."""
    ones_col, ones_row, eps_t = consts
    inv_d = 1.0 / (n_dt * P)
    for tci in range(tn // chunk):
        srcs = [src_of(dt, tci) for dt in range(n_dt)]   # f32r tiles
        sum_ps = lnp.tile([1, chunk], F32, tag="ln_sum", bufs=2)
        for dt in range(n_dt):
            nc.tensor.matmul(sum_ps, ones_col, srcs[dt],
                             start=(dt == 0), stop=(dt == n_dt - 1))
        sumsq_ps = lnp.tile([1, chunk], F32, tag="ln_sumsq", bufs=2)
        for dt in range(n_dt):
            sq_t = sbp.tile([P, chunk], F32R, tag="ln_sq", bufs=2)
            nc.scalar.activation(sq_t, srcs[dt].bitcast(F32), AF.Square)
            nc.tensor.matmul(sumsq_ps, ones_col, sq_t,
                             start=(dt == 0), stop=(dt == n_dt - 1))
        mu_row = sbp.tile([1, chunk], F32R, tag="ln_mu", bufs=2)
        nc.scalar.mul(out=mu_row, in_=sum_ps, mul=inv_d)
        var_row = sbp.tile([1, chunk], F32, tag="ln_var", bufs=2)
        nc.scalar.mul(out=var_row, in_=sumsq_ps, mul=inv_d)
        musq = sbp.tile([1, chunk], F32, tag="ln_musq", bufs=2)
        nc.vector.tensor_mul(out=musq, in0=mu_row.bitcast(F32),
                             in1=mu_row.bitcast(F32))
        nc.vector.tensor_tensor(out=var_row, in0=var_row, in1=musq,
                                op=OP.subtract)
        nc.scalar.activation(var_row, var_row, AF.Sqrt, bias=eps_t)
        rstd_row = sbp.tile([1, chunk], F32R, tag="ln_rstd", bufs=2)
        with nc.allow_low_precision("f32r keeps full fp32 bits here"):
            nc.vector.reciprocal(rstd_row, var_row)
        mu_b = lnp.tile([P, chunk], F32, tag="ln_mub", bufs=1)
        nc.tensor.matmul(mu_b, ones_row, mu_row, start=True, stop=True)
        rstd_b = lnp.tile([P, chunk], F32, tag="ln_rstdb", bufs=1)
        nc.tensor.matmul(rstd_b, ones_row, rstd_row, start=True, stop=True)
        mu_s = sbp.tile([P, chunk], F32, tag="ln_mus", bufs=2)
        nc.scalar.copy(out=mu_s, in_=mu_b)
        for dt in range(n_dt):
            tmp = sbp.tile([P, chunk], F32, tag="ln_tmp", bufs=3)
            eng = nc.gpsimd if dt % 2 else nc.vector
            src_in = mu_s if dt % 2 else mu_b
            eng.tensor_tensor(out=tmp, in0=srcs[dt].bitcast(F32),
                              in1=src_in, op=OP.subtract)
            nc.vector.tensor_tensor(out=dst_of(dt, tci), in0=tmp, in1=rstd_b,
                                    op=OP.mult)
        if post_cb is not None:
            post_cb(tci)


def build(flags):
    has_qkv1b, has_bo1, has_q2b, has_bo2, has_gegb, has_ffb = flags
    nc = bacc.Bacc()

    xT = nc.dram_tensor("xT", [D, N], F32R, kind="ExternalInput")
    xoT = nc.dram_tensor("xoT", [D, TO], F32R, kind="ExternalInput")
    ctxT = nc.dram_tensor("ctxT", [CD, CN], BF16, kind="ExternalInput")
    wq1T = nc.dram_tensor("wq1T", [D, D], BF16, kind="ExternalInput")
    wk1T = nc.dram_tensor("wk1T", [D, D], BF16, kind="ExternalInput")
    wv1T = nc.dram_tensor("wv1T", [D, D], BF16, kind="ExternalInput")
    wo1T = nc.dram_tensor("wo1T", [D, D], BF16, kind="ExternalInput")
    wq2T = nc.dram_tensor("wq2T", [D, D], BF16, kind="ExternalInput")
    wk2T = nc.dram_tensor("wk2T", [CD, D], BF16, kind="ExternalInput")
    wv2T = nc.dram_tensor("wv2T", [CD, D], BF16, kind="ExternalInput")
    wo2T = nc.dram_tensor("wo2T", [D, D], BF16, kind="ExternalInput")
    wgT = nc.dram_tensor("wgT", [D, 2 * FF], BF16, kind="ExternalInput")
    wfT = nc.dram_tensor("wfT", [FF, D], BF16, kind="ExternalInput")
    onesc = nc.dram_tensor("onesc", [P, 1], F32R, kind="ExternalInput")
    onesr = nc.dram_tensor("onesr", [1, P], F32R, kind="ExternalInput")
    onesb = nc.dram_tensor("onesb", [1, 512], BF16, kind="ExternalInput")
    bias_rows = {}
    if has_qkv1b:
        for nm in ("bq1", "bk1", "bv1"):
            bias_rows[nm] = nc.dram_tensor(nm, [1, D], BF16, kind="ExternalInput")
    if has_bo1:
        bias_rows["bo1"] = nc.dram_tensor("bo1", [1, D], BF16, kind="ExternalInput")
    if has_q2b:
        bias_rows["bq2"] = nc.dram_tensor("bq2", [1, D], BF16, kind="ExternalInput")
    if has_bo2:
        bias_rows["bo2"] = nc.dram_tensor("bo2", [1, D], BF16, kind="ExternalInput")
    if has_gegb:
        bias_rows["bgeg"] = nc.dram_tensor("bgeg", [1, 2 * FF], BF16,
                                           kind="ExternalInput")
    if has_ffb:
        bias_rows["bff"] = nc.dram_tensor("bff", [1, D], BF16, kind="ExternalInput")
    yT = nc.dram_tensor("yT", [D, TO], F32R, kind="ExternalOutput")

    xT_v = xT.rearrange("(dt p) t -> dt p t", p=P)
    xoT_v = xoT.rearrange("(dt p) t -> dt p t", p=P)
    ctxT_v = ctxT.rearrange("(ct p) t -> ct p t", p=P)
    yT_v = yT.rearrange("(dt p) t -> p dt t", p=P)

    def wview(w):
        return w.rearrange("(it p) o -> p it o", p=P)

    with tile.TileContext(nc) as tc:
        with tc.tile_pool(name="consts", bufs=1) as cpool, \
             tc.tile_pool(name="pers", bufs=1) as pers, \
             tc.tile_pool(name="wmain", bufs=1) as wmain:

            ones_col = cpool.tile([P, 1], F32R)
            nc.sync.dma_start(ones_col, onesc[:])
            ones_row = cpool.tile([1, P], F32R)
            nc.sync.dma_start(ones_row, onesr[:])
            ones_b = cpool.tile([1, 512], BF16)
            nc.sync.dma_start(ones_b, onesb[:])
            eps_t = cpool.tile([1, 1], F32)
            nc.vector.memset(eps_t, EPS)
            consts = (ones_col, ones_row, eps_t)

            bias_sb = {}
            for nm, t in bias_rows.items():
                bt = cpool.tile([1, t.shape[1]], BF16, tag=f"bias_{nm}")
                nc.sync.dma_start(bt, t[:])
                bias_sb[nm] = bt

            def proj_feature_major(pp, w_sb, act, out_cb, n_in, n_tok,
                                   bias=None, tag="pp256"):
                """out[oc] = sum_it w.T @ act; out_cb(oc, psum)."""
                for oc in range(DT):
                    ps = pp.tile([P, n_tok], F32, tag=tag, bufs=2)
                    for it in range(n_in):
                        nc.tensor.matmul(ps, w_sb[:, it, oc * P:(oc + 1) * P],
                                         act[:, it, :],
                                         start=(it == 0),
                                         stop=(it == n_in - 1 and bias is None))
                    if bias is not None:
                        nc.tensor.matmul(ps, bias[:, oc * P:(oc + 1) * P],
                                         ones_b[:, :n_tok], start=False,
                                         stop=True)
                    out_cb(oc, ps)

            x_ownT = pers.tile([P, DT, TO], F32R)      # residual stream (own)
            for dt in range(DT):
                nc.sync.dma_start(x_ownT[:, dt, :], xoT_v[dt])

            # cross-attn K2/V2 depend only on the context: computed early in
            # phase B so they overlap everything up to phase E.
            K2_sb = pers.tile([P, DT, CN], BF16)
            V2_sb = pers.tile([P, H, 65], BF16)

            # ========== attn1 scope: phases A-D ==========
            with tc.tile_pool(name="c1", bufs=1) as c1:
                O_sb = c1.tile([P, DT, TO], BF16)
                K_sb = c1.tile([P, DT, N], BF16)
                V_sb = c1.tile([P, NKT, H, 65], BF16)
                Q_sb = c1.tile([P, DT, TO], BF16)
                lnoT = c1.tile([P, DT, TO], BF16)

                with tc.tile_pool(name="c2", bufs=1) as c2:
                    ln1T = c2.tile([P, DT, N], BF16)

                    # ----- Phase A+B fused: LN1 chunk -> K/V for that chunk --
                    scopeA = nc.enter_named_scope("phA_ln1", False)
                    LCH = 512
                    # weight prefetches first: DMA runs under LN compute
                    wq1_sb = wmain.tile([P, DT, D], BF16, tag="w2m", bufs=2)
                    nc.sync.dma_start(wq1_sb, wview(wq1T))
                    wk1_sb = wmain.tile([P, DT, D], BF16, tag="w2m", bufs=2)
                    nc.sync.dma_start(wk1_sb, wview(wk1T))
                    bk1 = bias_sb.get("bk1")
                    bv1 = bias_sb.get("bv1")
                    with tc.tile_pool(name="lnps", bufs=2, space="PSUM") as lnp, \
                         tc.tile_pool(name="lnsb", bufs=2) as lnsb:
                        with tc.tile_pool(name="qps", bufs=2,
                                          space="PSUM") as qpp:
                            _ln_feature_major(
                                nc, lnp, lnsb, consts,
                                lambda dt, tci: x_ownT[:, dt, :],
                                lambda dt, tci: lnoT[:, dt, :],
                                DT, TO, TO)
                            proj_feature_major(
                                qpp, wq1_sb, lnoT,
                                lambda oc, ps: nc.scalar.copy(
                                    out=Q_sb[:, oc, :], in_=ps),
                                DT, TO, bias=bias_sb.get("bq1"))
                        nc.vector.memset(V_sb, 1.0)
                        with tc.tile_pool(name="projps", bufs=2,
                                          space="PSUM") as pp:
                            wv1_sb = wmain.tile([P, DT, D], BF16, tag="w2m",
                                                bufs=2)
                            nc.sync.dma_start(wv1_sb, wview(wv1T))

                            def kv_for_chunk(tci):
                                for oc in range(DT):
                                    k_ps = pp.tile([P, 512], F32, tag="pp512",
                                                   bufs=2)
                                    for it in range(DT):
                                        nc.tensor.matmul(
                                            k_ps,
                                            wk1_sb[:, it, oc * P:(oc + 1) * P],
                                            ln1T[:, it,
                                                 tci * 512:(tci + 1) * 512],
                                            start=(it == 0),
                                            stop=(it == DT - 1 and bk1 is None))
                                    if bk1 is not None:
                                        nc.tensor.matmul(
                                            k_ps, bk1[:, oc * P:(oc + 1) * P],
                                            ones_b, start=False, stop=True)
                                    nc.vector.tensor_copy(
                                        out=K_sb[:, oc,
                                                 tci * 512:(tci + 1) * 512],
                                        in_=k_ps)
                                for kt in range(tci * 4, tci * 4 + 4):
                                    for hc in range(2):
                                        v_ps = pp.tile([P, 512], F32,
                                                       tag="pp512", bufs=2)
                                        for it in range(DT):
                                            nc.tensor.matmul(
                                                v_ps,
                                                ln1T[:, it, kt * P:(kt + 1) * P],
                                                wv1_sb[:, it,
                                                       hc * 512:(hc + 1) * 512],
                                                start=(it == 0),
                                                stop=(it == DT - 1
                                                      and bv1 is None))
                                        if bv1 is not None:
                                            nc.tensor.matmul(
                                                v_ps, ones_row.bitcast(BF16),
                                                bv1[:, hc * 512:(hc + 1) * 512],
                                                start=False, stop=True)
                                        nc.scalar.copy(
                                            out=V_sb[:, kt,
                                                     hc * 8:(hc + 1) * 8, 0:64],
                                            in_=v_ps.rearrange(
                                                "p (h d) -> p h d", d=64))

                            def load_x(dt, tci, _c={}):
                                if (dt, tci) not in _c:
                                    t = lnsb.tile([P, LCH], F32R, tag="xt",
                                                  bufs=9)
                                    nc.sync.dma_start(
                                        t,
                                        xT_v[dt, :, tci * LCH:(tci + 1) * LCH])
                                    _c[(dt, tci)] = t
                                return _c[(dt, tci)]

                            _ln_feature_major(
                                nc, lnp, lnsb, consts, load_x,
                                lambda dt, tci: ln1T[:, dt,
                                                     tci * LCH:(tci + 1) * LCH],
                                DT, N, LCH, post_cb=kv_for_chunk)
                    nc.leave_named_scope("phA_ln1", scopeA[0], False)

                    # ----- Phase B remainder: K2/V2 (context) -----
                    scopeB = nc.enter_named_scope("phB_qkv", False)
                    with tc.tile_pool(name="wb", bufs=1) as wpool, \
                         tc.tile_pool(name="projps2", bufs=2, space="PSUM") as pp:
                        # K2/V2 from context (independent of x)
                        ctx_sb = wpool.tile([P, CT, CN], BF16, tag="ctx", bufs=1)
                        for ct in range(CT):
                            nc.sync.dma_start(ctx_sb[:, ct, :], ctxT_v[ct])
                        wk2_sb = wpool.tile([P, CT, D], BF16, tag="w15", bufs=2)
                        nc.sync.dma_start(wk2_sb, wview(wk2T))
                        for oc in range(DT):
                            k_ps = pp.tile([P, CN], F32, tag="ppsm", bufs=2)
                            for it in range(CT):
                                nc.tensor.matmul(
                                    k_ps, wk2_sb[:, it, oc * P:(oc + 1) * P],
                                    ctx_sb[:, it, :],
                                    start=(it == 0), stop=(it == CT - 1))
                            nc.scalar.copy(out=K2_sb[:, oc, :], in_=k_ps)
                        nc.vector.memset(V2_sb, 1.0)
                        wv2_sb = wpool.tile([P, CT, D], BF16, tag="w15", bufs=2)
                        nc.sync.dma_start(wv2_sb, wview(wv2T))
                        for hc in range(2):
                            v_ps = pp.tile([CN, 512], F32, tag="ppsm", bufs=2)
                            for it in range(CT):
                                nc.tensor.matmul(
                                    v_ps, ctx_sb[:, it, :],
                                    wv2_sb[:, it, hc * 512:(hc + 1) * 512],
                                    start=(it == 0), stop=(it == CT - 1))
                            nc.scalar.copy(
                                out=V2_sb[0:CN, hc * 8:(hc + 1) * 8, 0:64],
                                in_=v_ps.rearrange("p (h d) -> p h d", d=64))
                    nc.leave_named_scope("phB_qkv", scopeB[0], False)

                # ----- Phase C: self-attention heads -----
                scopeC = nc.enter_named_scope("phC_attn", False)
                with tc.tile_pool(name="aps", bufs=1, space="PSUM") as apsum, \
                     tc.tile_pool(name="asb", bufs=1) as asb:
                    for h in range(H):
                        j, r0 = h >> 1, (h & 1) * 64
                        o_ps = apsum.tile([65, TO], F32, tag="o_ps", bufs=3)
                        for kt2 in range(NKT // 2):
                            # two key-tiles share one PSUM bank; second matmul
                            # overwrites its (cleared) half via has_written
                            s_ps = apsum.tile([P, 2, TO], F32, tag="s_ps", bufs=3)
                            for half in range(2):
                                kt = kt2 * 2 + half
                                nc.tensor.matmul(
                                    s_ps[:, half, :],
                                    K_sb[r0:r0 + 64, j, kt * P:(kt + 1) * P],
                                    Q_sb[r0:r0 + 64, j, :],
                                    start=(half == 0), stop=True,
                                    skip_group_check=(half == 1))
                            e_t = asb.tile([P, 2, TO], BF16, tag="e_t", bufs=6)
                            nc.scalar.activation(e_t, s_ps, AF.Exp, scale=SCALE)
                            for half in range(2):
                                kt = kt2 * 2 + half
                                nc.tensor.matmul(
                                    o_ps, V_sb[:, kt, h, :], e_t[:, half, :],
                                    start=(kt == 0), stop=(kt == NKT - 1))
                        r_sb = asb.tile([1, TO], F32R, tag="r_sb", bufs=4)
                        with nc.allow_low_precision("f32r == f32 bits"):
                            nc.vector.reciprocal(r_sb, o_ps[64:65, :])
                        r_ps = apsum.tile([64, TO], F32, tag="r_ps", bufs=2)
                        nc.tensor.matmul(r_ps, ones_row[:, :64], r_sb,
                                         start=True, stop=True)
                        r_bc = asb.tile([64, TO], F32, tag="r_bc", bufs=3)
                        nc.scalar.copy(out=r_bc, in_=r_ps)
                        nc.vector.tensor_tensor(out=O_sb[r0:r0 + 64, j, :],
                                                in0=o_ps[0:64, :],
                                                in1=r_bc, op=OP.mult)
                nc.leave_named_scope("phC_attn", scopeC[0], False)

                # ----- Phase D: attn1 out-proj + residual -----
                scopeD = nc.enter_named_scope("phD_oproj", False)
                with tc.tile_pool(name="dps", bufs=3, space="PSUM") as pp:
                    wo1_sb = wmain.tile([P, DT, D], BF16, tag="w2m", bufs=2)
                    nc.sync.dma_start(wo1_sb, wview(wo1T))

                    def add_residual(oc, ps):
                        nc.vector.tensor_tensor(
                            out=x_ownT[:, oc, :],
                            in0=x_ownT[:, oc, :].bitcast(F32),
                            in1=ps, op=OP.add)

                    proj_feature_major(pp, wo1_sb, O_sb, add_residual, DT, TO,
                                       bias=bias_sb.get("bo1"))
                nc.leave_named_scope("phD_oproj", scopeD[0], False)

            # ========== attn2 scope: phase E ==========
            scopeE = nc.enter_named_scope("phE_xattn", False)
            with tc.tile_pool(name="ce", bufs=1) as ce:
                ln2T = ce.tile([P, DT, TO], BF16)
                Q2_sb = ce.tile([P, DT, TO], BF16)
                O2_sb = ce.tile([P, DT, TO], BF16)

                with tc.tile_pool(name="lnps2", bufs=2, space="PSUM") as lnp, \
                     tc.tile_pool(name="lnsb2", bufs=2) as lnsb:
                    _ln_feature_major(
                        nc, lnp, lnsb, consts,
                        lambda dt, tci: x_ownT[:, dt, :],
                        lambda dt, tci: ln2T[:, dt, :],
                        DT, TO, TO)

                with tc.tile_pool(name="eps_", bufs=2, space="PSUM") as pp:
                    wq2_sb = wmain.tile([P, DT, D], BF16, tag="w2m", bufs=2)
                    nc.sync.dma_start(wq2_sb, wview(wq2T))
                    proj_feature_major(
                        pp, wq2_sb, ln2T,
                        lambda oc, ps: nc.scalar.copy(out=Q2_sb[:, oc, :],
                                                      in_=ps),
                        DT, TO, bias=bias_sb.get("bq2"))

                with tc.tile_pool(name="aps2", bufs=1, space="PSUM") as apsum, \
                     tc.tile_pool(name="asb2", bufs=1) as asb:
                    for h in range(H):
                        j, r0 = h >> 1, (h & 1) * 64
                        s_ps = apsum.tile([CN, TO], F32, tag="s_ps", bufs=3)
                        nc.tensor.matmul(
                            s_ps, K2_sb[r0:r0 + 64, j, :],
                            Q2_sb[r0:r0 + 64, j, :], start=True, stop=True)
                        e_t = asb.tile([CN, TO], BF16, tag="e_t", bufs=4)
                        nc.scalar.activation(e_t, s_ps, AF.Exp, scale=SCALE)
                        o_ps = apsum.tile([65, TO], F32, tag="o_ps", bufs=2)
                        nc.tensor.matmul(o_ps, V2_sb[0:CN, h, :], e_t,
                                         start=True, stop=True)
                        r_sb = asb.tile([1, TO], F32R, tag="r_sb", bufs=4)
                        with nc.allow_low_precision("f32r == f32 bits"):
                            nc.vector.reciprocal(r_sb, o_ps[64:65, :])
                        r_ps = apsum.tile([64, TO], F32, tag="r_ps", bufs=2)
                        nc.tensor.matmul(r_ps, ones_row[:, :64], r_sb,
                                         start=True, stop=True)
                        r_bc = asb.tile([64, TO], F32, tag="r_bc", bufs=3)
                        nc.scalar.copy(out=r_bc, in_=r_ps)
                        nc.vector.tensor_tensor(out=O2_sb[r0:r0 + 64, j, :],
                                                in0=o_ps[0:64, :],
                                                in1=r_bc, op=OP.mult)

                with tc.tile_pool(name="eps2", bufs=3, space="PSUM") as pp:
                    wo2_sb = wmain.tile([P, DT, D], BF16, tag="w2m", bufs=2)
                    nc.sync.dma_start(wo2_sb, wview(wo2T))

                    def add_residual2(oc, ps):
                        nc.vector.tensor_tensor(
                            out=x_ownT[:, oc, :],
                            in0=x_ownT[:, oc, :].bitcast(F32),
                            in1=ps, op=OP.add)

                    proj_feature_major(pp, wo2_sb, O2_sb, add_residual2, DT, TO,
                                       bias=bias_sb.get("bo2"))
            nc.leave_named_scope("phE_xattn", scopeE[0], False)

            # ========== FFN scope: phase F ==========
            scopeF = nc.enter_named_scope("phF_ffn", False)
            with tc.tile_pool(name="cf", bufs=1) as cf:
                ln3T = cf.tile([P, DT, TO], BF16)
                Hbuf = cf.tile([P, FT, TO], BF16)

                with tc.tile_pool(name="lnps3", bufs=2, space="PSUM") as lnp, \
                     tc.tile_pool(name="lnsb3", bufs=2) as lnsb:
                    _ln_feature_major(
                        nc, lnp, lnsb, consts,
                        lambda dt, tci: x_ownT[:, dt, :],
                        lambda dt, tci: ln3T[:, dt, :],
                        DT, TO, TO)

                wgT_v = wview(wgT)
                bgeg = bias_sb.get("bgeg")
                with tc.tile_pool(name="wg", bufs=1) as wgpool, \
                     tc.tile_pool(name="gps", bufs=1, space="PSUM") as gpsum, \
                     tc.tile_pool(name="gsb", bufs=3) as gsb:
                    for g in range(8):
                        wg_h = wgpool.tile([P, DT, 512], BF16, tag="wgh", bufs=2)
                        nc.sync.dma_start(wg_h,
                                          wgT_v[:, :, g * 512:(g + 1) * 512])
                        wg_g = wgpool.tile([P, DT, 512], BF16, tag="wgg", bufs=2)
                        nc.sync.dma_start(
                            wg_g, wgT_v[:, :, FF + g * 512:FF + (g + 1) * 512])
                        for fi in range(4):
                            f = g * 4 + fi
                            h_ps = gpsum.tile([P, TO], F32, tag="h_ps", bufs=2)
                            for it in range(DT):
                                nc.tensor.matmul(
                                    h_ps, wg_h[:, it, fi * P:(fi + 1) * P],
                                    ln3T[:, it, :],
                                    start=(it == 0),
                                    stop=(it == DT - 1 and bgeg is None))
                            if bgeg is not None:
                                nc.tensor.matmul(
                                    h_ps, bgeg[:, f * P:(f + 1) * P],
                                    ones_b[:, :TO], start=False, stop=True)
                            g_ps = gpsum.tile([P, TO], F32, tag="g_ps", bufs=2)
                            for it in range(DT):
                                nc.tensor.matmul(
                                    g_ps, wg_g[:, it, fi * P:(fi + 1) * P],
                                    ln3T[:, it, :],
                                    start=(it == 0),
                                    stop=(it == DT - 1 and bgeg is None))
                            if bgeg is not None:
                                nc.tensor.matmul(
                                    g_ps,
                                    bgeg[:, FF + f * P:FF + (f + 1) * P],
                                    ones_b[:, :TO], start=False, stop=True)
                            gel = gsb.tile([P, TO], F32, tag="gel", bufs=3)
                            nc.scalar.activation(gel, g_ps, AF.Gelu)
                            nc.vector.tensor_tensor(out=Hbuf[:, f, :],
                                                    in0=h_ps, in1=gel,
                                                    op=OP.mult)

                # ffout: two-level accumulation; spills add into x_ownT
                wfT_v = wfT.rearrange("(f p) o -> f p o", p=P)
                bff = bias_sb.get("bff")
                with tc.tile_pool(name="wfp", bufs=1) as wfpool, \
                     tc.tile_pool(name="yps", bufs=2, space="PSUM") as yp_:
                    for fg in range(4):
                        wf_tiles = []
                        for f8 in range(8):
                            wt = wfpool.tile([P, D], BF16, tag="wft", bufs=10)
                            nc.sync.dma_start(wt, wfT_v[fg * 8 + f8])
                            wf_tiles.append(wt)
                        for oc in range(DT):
                            i_ps = yp_.tile([P, TO], F32, tag="i_ps")
                            add_bias = bff is not None and fg == 3
                            for f8 in range(8):
                                nc.tensor.matmul(
                                    i_ps, wf_tiles[f8][:, oc * P:(oc + 1) * P],
                                    Hbuf[:, fg * 8 + f8, :],
                                    start=(f8 == 0),
                                    stop=(f8 == 7 and not add_bias))
                            if add_bias:
                                nc.tensor.matmul(
                                    i_ps, bff[:, oc * P:(oc + 1) * P],
                                    ones_b[:, :TO], start=False, stop=True)
                            nc.vector.tensor_tensor(
                                out=x_ownT[:, oc, :],
                                in0=x_ownT[:, oc, :].bitcast(F32),
                                in1=i_ps, op=OP.add)
                            if fg == 3:
                                nc.sync.dma_start(yT_v[:, oc, :],
                                                  x_ownT[:, oc, :])
            nc.leave_named_scope("phF_ffn", scopeF[0], False)

    nc.finalize()
    return nc


_CACHE = {}


def kernel(**inputs):
    def f32c(a):
        return np.ascontiguousarray(np.asarray(a, dtype=np.float32))

    def bfT(w):
        """W [out,in] (optionally gain-folded) -> bf16 W.T contiguous."""
        return np.ascontiguousarray(w.T).astype(ml_dtypes.bfloat16)

    x = f32c(inputs["hidden_states"])[0]          # [N, D]
    ctx = f32c(inputs["context"])[0]              # [CN, CD]
    g1 = f32c(inputs["ln1_g"]); b1 = f32c(inputs["ln1_b"])
    g2 = f32c(inputs["ln2_g"]); b2 = f32c(inputs["ln2_b"])
    g3 = f32c(inputs["ln3_g"]); b3 = f32c(inputs["ln3_b"])
    wq1 = f32c(inputs["wq1"]); wk1 = f32c(inputs["wk1"]); wv1 = f32c(inputs["wv1"])
    wo1 = f32c(inputs["wo1"]); bo1 = f32c(inputs["bo1"])
    wq2 = f32c(inputs["wq2"]); wk2 = f32c(inputs["wk2"]); wv2 = f32c(inputs["wv2"])
    wo2 = f32c(inputs["wo2"]); bo2 = f32c(inputs["bo2"])
    wg = f32c(inputs["w_geglu"]); bg = f32c(inputs["b_geglu"])
    wf = f32c(inputs["w_ffout"]); bf = f32c(inputs["b_ffout"])

    bq1 = wq1 @ b1; bk1 = wk1 @ b1; bv1 = wv1 @ b1
    bq2 = wq2 @ b2
    bgeg = bg + wg @ b3
    flags = (bool(np.any(bq1) or np.any(bk1) or np.any(bv1)), bool(np.any(bo1)),
             bool(np.any(bq2)), bool(np.any(bo2)), bool(np.any(bgeg)),
             bool(np.any(bf)))

    if flags not in _CACHE:
        _CACHE[flags] = build(flags)
    nc = _CACHE[flags]

    xT = np.ascontiguousarray(x.T)                # [D, N]
    bf16 = ml_dtypes.bfloat16
    shared = {
        "xT": xT,
        "ctxT": np.ascontiguousarray(ctx.T).astype(bf16),
        "wq1T": bfT(wq1 * g1[None, :]),
        "wk1T": bfT(wk1 * g1[None, :]),
        "wv1T": bfT(wv1 * g1[None, :]),
        "wo1T": bfT(wo1),
        "wq2T": bfT(wq2 * g2[None, :]),
        "wk2T": bfT(wk2),
        "wv2T": bfT(wv2),
        "wo2T": bfT(wo2),
        "wgT": bfT(wg * g3[None, :]),
        "wfT": bfT(wf),
        "onesc": np.ones((P, 1), np.float32),
        "onesr": np.ones((1, P), np.float32),
        "onesb": np.ones((1, 512), bf16),
    }
    if flags[0]:
        shared["bq1"] = bq1[None, :].astype(bf16)
        shared["bk1"] = bk1[None, :].astype(bf16)
        shared["bv1"] = bv1[None, :].astype(bf16)
    if flags[1]:
        shared["bo1"] = bo1[None, :].astype(bf16)
    if flags[2]:
        shared["bq2"] = bq2[None, :].astype(bf16)
    if flags[3]:
        shared["bo2"] = bo2[None, :].astype(bf16)
    if flags[4]:
        shared["bgeg"] = bgeg[None, :].astype(bf16)
    if flags[5]:
        shared["bff"] = bf[None, :].astype(bf16)

    in_maps = []
    for c in range(NCORES):
        m = dict(shared)
        m["xoT"] = np.ascontiguousarray(xT[:, c * TO:(c + 1) * TO])
        in_maps.append(m)

    res = run_bass_kernel_spmd(nc, in_maps, core_ids=list(range(NCORES)))
    yT = np.concatenate([r["yT"] for r in res.results], axis=1)  # [D, N]
    return np.ascontiguousarray(yT.T)[None].astype(np.float32)
